# revision 2
# baseline (speedup 1.0000x reference)
"""Trainium2 Bass kernel for nn_Attn_40046275068166.

Tiny causal MHA over huge batch: x[B=65536, T=34, D=6], 2 heads, head_dim 3.
Strategy: pure data parallelism over 8 cores (batch sharded), batch on the
128 SBUF partitions inside each core. All per-example compute is expressed
as DVE tensor ops with broadcast access patterns; exp runs on the scalar
engine (ACT). Software-pipelined: phase A (projections + scores) of tile n
overlaps ACT exp of tile n-1 and phase B (softmax-normalize + PV + output
projection) of tile n-1. Raw bass (no Tile framework) with explicit
semaphores — this walrus build allows at most one sync-wait per instruction,
so every multi-dependency is expressed as standalone wait ops.

Perf: the score/softmax/PV datapath is fp16 so that every tensor_tensor
(add/mul) with unit-stride innermost dims hits the DVE 2x perf mode.
Reductions and the softmax denominator stay fp32.

Math identity used to skip separate q/k projections:
  s[b,h,i,j] = q_i . k_j / sqrt(hd) = xp_i^T A_h xp_j,  A_h = Wq_h^T Wk_h/sqrt(hd)
so only y = A_h xp (per j) and v = Wv xt are projected, and s = xp_i . y_j.
Causal mask applied additively (-30000, fp16-safe) before exp.
"""

import math
from contextlib import ExitStack
from functools import lru_cache

import numpy as np

import concourse.bass as bass
from concourse import mybir
from concourse.bass_utils import run_bass_kernel_spmd

NCORES = 8
T = 34
D = 6
NH = 2
HD = 3
POS = 3
TT = T * T          # 1156
STT = NH * TT       # 2312 score elems per example
P = 128

F32 = mybir.dt.float32
F16 = mybir.dt.float16

# fp32 constants vector layout (element offsets)
OFF_A2 = 0          # [2][6][3]  w=0: y-proj weights, w=1: v-proj weights
CLEN = 36

# fp16 constants vector layout
OFFH_MASK = 0       # [1156]     additive causal mask (0 / -30000)
OFFH_WO = TT        # [6][6]     WoM[dm][e]
HLEN = TT + 36

MASKV = -30000.0    # fits fp16; exp() underflows to exactly 0


def _ap(t, off, dims):
    """AP on SBUF tensor t: explicit free dims [(stride, count), ...]."""
    p0 = t[:].ap[0]
    return bass.AP(tensor=t, offset=off, ap=[list(p0)] + [list(d) for d in dims])


def build_kernel(bc, G):
    """bc: per-core batch, G: b-groups of 128 per pipeline tile."""
    assert bc % (P * G) == 0
    NT = bc // (P * G)
    GT = G * T * D          # x elements per partition per set (g,t,d)
    SC = G * STT            # score elems per partition per set

    nc = bass.Bass("TRN2")
    x = nc.dram_tensor("x", [bc, T, D], F32, kind="ExternalInput")
    wts = nc.dram_tensor("wts", [CLEN], F32, kind="ExternalInput")
    wth = nc.dram_tensor("wth", [HLEN], F16, kind="ExternalInput")
    out = nc.dram_tensor("out", [bc, T, D], F32, kind="ExternalOutput")

    xr = x[:].rearrange("(n g p) t d -> n p g t d", g=G, p=P)
    outr = out[:].rearrange("(n g p) t d -> n p g t d", g=G, p=P)
    wts_b = bass.AP(tensor=wts, offset=0, ap=[[0, P], [1, CLEN]])
    wth_b = bass.AP(tensor=wth, offset=0, ap=[[0, P], [1, HLEN]])

    with ExitStack() as ctx:
        sb = lambda nm, shape, dt=F32: ctx.enter_context(
            nc.sbuf_tensor(nm, shape, dt))
        wsb = sb("wsb", [P, CLEN])
        wsh = sb("wsh", [P, HLEN], F16)
        xin = sb("xin", [P, 2, G, T, D])
        # [set][w][g][hc][j] — j innermost (stride 1): DVE broadcast reads
        # with non-unit inner strides cost ~1.7x; stride-0/1 run at full rate
        yv = sb("yv", [P, 2, 2, G, D, T], F16)
        pp = sb("pp", [P, G, NH, T, T], F16)   # PV products (dead block stays 0)
        t0 = sb("t0", [P, 2, G, NH, T, T], F16)
        t1 = sb("t1", [P, 2, G, NH, T, T], F16)
        tmp = sb("tmp", [P, T, D], F16)
        den = sb("den", [P, G, NH, T])
        rcp = sb("rcp", [P, G, NH, T])
        o2 = sb("o2", [P, G, T, D])           # [g][t][e=(h,c)]  fp32
        o2b = sb("o2b", [P, G, T, D], F16)    # normalized, fp16
        prod = sb("prod", [P, G, D, T, D], F16)   # [g][dm][t][e]
        res = sb("res", [P, 2, G, T, D])

        # dma_in/out_sem are parity-split: a DMA's 16 per-engine +1s only
        # certify completion if no OTHER DMA on the same semaphore is in
        # flight (8 engines finishing two DMAs also reads as "16"). With
        # even/odd semaphores plus the xin_done/res_done gating, at most one
        # DMA per semaphore is outstanding when a wait on it passes.
        sem_names = ["dma_in0", "dma_in1", "const", "xin_done", "s_done",
                     "e_done", "b_done", "res_done", "out0", "out1"]
        sems = {k: ctx.enter_context(nc.semaphore(name=k)) for k in sem_names}

        # element strides within a partition
        XIN_SET = G * T * D
        XIN_G = T * D
        YV_SET = 2 * G * T * D
        YV_W = G * T * D
        YV_G = T * D
        TS_SET = G * NH * TT            # t0/t1 set stride
        TS_G = NH * TT
        TS_H = TT

        block = ctx.enter_context(nc.Block())

        @block.gpsimd
        def _(sync):
            # SWDGE (software DGE): exactly one +16 sem increment per
            # dma_start on completion. HWDGE (nc.sync) fans a DMA out over
            # several hardware queues, each of which bumps the semaphore by
            # 16 — a single >=16*(n+1) wait would then fire before the whole
            # transfer has landed.
            def store(k):
                sp = k % 2
                sync.wait_ge(sems["res_done"], k + 1)
                sync.dma_start(
                    out=outr[k],
                    in_=_ap(res, sp * XIN_SET, [(XIN_G, G), (1, T * D)]),
                ).then_inc(sems["out0" if sp == 0 else "out1"], 16)

            sync.dma_start(out=wsb[:], in_=wts_b).then_inc(sems["const"], 16)
            sync.dma_start(out=wsh[:], in_=wth_b).then_inc(sems["const"], 16)
            for n in range(NT):
                s = n % 2
                if n >= 2:
                    sync.wait_ge(sems["xin_done"], n - 1)
                sync.dma_start(
                    out=_ap(xin, s * XIN_SET, [(XIN_G, G), (1, T * D)]),
                    in_=xr[n],
                ).then_inc(sems["dma_in0" if s == 0 else "dma_in1"], 16)
                # store lags two tiles so its res_done wait never blocks the
                # next load's descriptor generation in this FIFO
                if n >= 2:
                    store(n - 2)
            store(NT - 2)
            store(NT - 1)
            # quiesce: don't let the program end with the last store in flight
            sync.wait_ge(sems["out0"], 16 * ((NT + 1) // 2))
            sync.wait_ge(sems["out1"], 16 * (NT // 2))

        @block.scalar
        def _(scalar):
            for n in range(NT):
                s = n % 2
                if n >= 2:
                    scalar.wait_ge(sems["b_done"], n - 1)
                scalar.wait_ge(sems["s_done"], n + 1)
                scalar.activation(
                    out=_ap(t1, s * TS_SET, [(1, SC)]),
                    in_=_ap(t0, s * TS_SET, [(1, SC)]),
                    func=mybir.ActivationFunctionType.Exp,
                ).then_inc(sems["e_done"], 1)

        @block.vector
        def _(vector):
            vector.wait_ge(sems["const"], 32)
            # Causal blocks over the TxT score plane (H = T//2):
            #   blk A: i<H,  j<H   (has diagonal -> mask)
            #   blk B: i>=H, j<H   (fully causal -> no mask)
            #   blk C: i>=H, j>=H  (has diagonal -> mask)
            # dead:   i<H,  j>=H   (never computed)
            # t0's dead block is set to MASKV once (exp -> 0); pp's dead block
            # to 0 once (reduce adds 0). Neither is ever rewritten.
            H = T // 2
            BLKS = [(0, 0), (H, 0), (H, H)]
            for s in range(2):
                vector.memset(
                    _ap(t0, s * TS_SET + H, [(TT, G * NH), (T, H), (1, T - H)]),
                    MASKV)
            vector.memset(_ap(pp, H, [(TT, G * NH), (T, H), (1, T - H)]), 0.0)

            def sadd(dst, dof, i0t, i0o, i1t, i1o):
                """dst[blocks] = in0 + in1 over AB (j<H) and C regions."""
                for (ro, li, lj) in ((0, T, H), (H * T + H, T - H, T - H)):
                    vector.tensor_add(
                        out=_ap(dst, dof + ro, [(TT, G * NH), (T, li), (1, lj)]),
                        in0=_ap(i0t, i0o + ro, [(TT, G * NH), (T, li), (1, lj)]),
                        in1=_ap(i1t, i1o + ro, [(TT, G * NH), (T, li), (1, lj)]))

            def phase_a(n):
                s = n % 2
                vector.wait_ge(sems["dma_in0" if s == 0 else "dma_in1"],
                               16 * (n // 2 + 1))
                # projections: yv[s, w, g, hc, j] = sum_b x[j, base+b]*A2[w,hc,b]
                for w in range(2):
                    for g in range(G):
                        xoff = s * XIN_SET + g * XIN_G + (3 - 3 * w)
                        yoff = s * YV_SET + w * YV_W + g * YV_G
                        for b in range(POS):
                            i0 = _ap(xin, xoff + b, [(0, D), (D, T)])
                            i1 = _ap(wsb, OFF_A2 + w * 18 + b, [(3, D), (0, T)])
                            if b == 0:
                                vector.tensor_mul(
                                    out=_ap(yv, yoff, [(T, D), (1, T)]),
                                    in0=i0, in1=i1)
                            else:
                                vector.tensor_mul(
                                    out=_ap(tmp, 0, [(T, D), (1, T)]),
                                    in0=i0, in1=i1)
                                vector.tensor_add(
                                    out=_ap(yv, yoff, [(1, T * D)]),
                                    in0=_ap(yv, yoff, [(1, T * D)]),
                                    in1=_ap(tmp, 0, [(1, T * D)]))
                # scores: t[g,h,i,j] = sum_a xp[g,i,a] * y[g,(h,a),j], blocked
                for a in range(POS):
                    dst = t0 if a == 0 else t1
                    for h in range(NH):
                        for bi, (i0b, j0b) in enumerate(BLKS):
                            li = H if i0b == 0 else T - H
                            lj = H if j0b == 0 else T - H
                            mm = vector.tensor_mul(
                                out=_ap(dst, s * TS_SET + h * TT + i0b * T + j0b,
                                        [(TS_G, G), (T, li), (1, lj)]),
                                in0=_ap(xin, s * XIN_SET + 3 + a + i0b * D,
                                        [(XIN_G, G), (D, li), (0, lj)]),
                                in1=_ap(yv, s * YV_SET + (h * HD + a) * T + j0b,
                                        [(YV_G, G), (0, li), (1, lj)]))
                            if a == POS - 1 and h == NH - 1 and bi == len(BLKS) - 1:
                                mm.then_inc(sems["xin_done"], 1)
                    if a == 1:
                        sadd(t0, s * TS_SET, t0, s * TS_SET, t1, s * TS_SET)
                # t1 += mask on diagonal blocks A and C; then t0 += t1
                for ro in (0, H * T + H):
                    vector.tensor_add(
                        out=_ap(t1, s * TS_SET + ro,
                                [(TT, G * NH), (T, H), (1, H)]),
                        in0=_ap(t1, s * TS_SET + ro,
                                [(TT, G * NH), (T, H), (1, H)]),
                        in1=_ap(wsh, OFFH_MASK + ro,
                                [(0, G * NH), (T, H), (1, H)]))
                vector.tensor_add(
                    out=_ap(t0, s * TS_SET, [(TT, G * NH), (T, T), (1, H)]),
                    in0=_ap(t0, s * TS_SET, [(TT, G * NH), (T, T), (1, H)]),
                    in1=_ap(t1, s * TS_SET, [(TT, G * NH), (T, T), (1, H)]))
                ro = H * T + H
                vector.tensor_add(
                    out=_ap(t0, s * TS_SET + ro,
                            [(TT, G * NH), (T, T - H), (1, T - H)]),
                    in0=_ap(t0, s * TS_SET + ro,
                            [(TT, G * NH), (T, T - H), (1, T - H)]),
                    in1=_ap(t1, s * TS_SET + ro,
                            [(TT, G * NH), (T, T - H), (1, T - H)])
                ).then_inc(sems["s_done"], 1)

            def phase_b(n):
                s = n % 2
                vector.wait_ge(sems["e_done"], n + 1)
                if n >= 2:
                    # WAR: res[s] still being read by out-DMA(n-2) (same parity)
                    vector.wait_ge(sems["out0" if s == 0 else "out1"],
                                   16 * (n // 2))
                # row sums over j (i<H reads only j<H), then reciprocal
                vector.tensor_reduce(
                    out=_ap(den, 0, [(NH * T, G), (T, NH), (1, H)]),
                    in_=_ap(t1, s * TS_SET, [(TT, G * NH), (T, H), (1, H)]),
                    axis=mybir.AxisListType.X, op=mybir.AluOpType.add)
                vector.tensor_reduce(
                    out=_ap(den, H, [(NH * T, G), (T, NH), (1, T - H)]),
                    in_=_ap(t1, s * TS_SET + H * T,
                            [(TT, G * NH), (T, T - H), (1, T)]),
                    axis=mybir.AxisListType.X, op=mybir.AluOpType.add)
                vector.reciprocal(
                    out=_ap(rcp, 0, [(1, G * NH * T)]),
                    in_=_ap(den, 0, [(1, G * NH * T)]))
                # PV: pp = e * v (blocked), then o2[g,i,(h,c)] = sum_j pp
                for c in range(HD):
                    for h in range(NH):
                        for bi, (i0b, j0b) in enumerate(BLKS):
                            li = H if i0b == 0 else T - H
                            lj = H if j0b == 0 else T - H
                            mm = vector.tensor_mul(
                                out=_ap(pp, h * TT + i0b * T + j0b,
                                        [(NH * TT, G), (T, li), (1, lj)]),
                                in0=_ap(t1, s * TS_SET + h * TT + i0b * T + j0b,
                                        [(TS_G, G), (T, li), (1, lj)]),
                                in1=_ap(yv, s * YV_SET + YV_W + (h * HD + c) * T
                                        + j0b, [(YV_G, G), (0, li), (1, lj)]))
                            if c == HD - 1 and h == NH - 1 and bi == len(BLKS) - 1:
                                mm.then_inc(sems["b_done"], 1)
                    vector.tensor_reduce(
                        out=_ap(o2, c, [(T * D, G), (HD, NH), (D, H)]),
                        in_=_ap(pp, 0, [(TT, G * NH), (T, H), (1, H)]),
                        axis=mybir.AxisListType.X, op=mybir.AluOpType.add)
                    vector.tensor_reduce(
                        out=_ap(o2, c + H * D,
                                [(T * D, G), (HD, NH), (D, T - H)]),
                        in_=_ap(pp, H * T,
                                [(TT, G * NH), (T, T - H), (1, T)]),
                        axis=mybir.AxisListType.X, op=mybir.AluOpType.add)
                # normalize: o2b = o2 * rcp (broadcast over c), fp16 out
                for g in range(G):
                    vector.tensor_mul(
                        out=_ap(o2b, g * T * D, [(D, T), (HD, NH), (1, HD)]),
                        in0=_ap(o2, g * T * D, [(D, T), (HD, NH), (1, HD)]),
                        in1=_ap(rcp, g * NH * T, [(1, T), (T, NH), (0, HD)]))
                # output projection: res[g,t,dm] = sum_e o2b[g,t,e]*WoM[dm,e]
                for g in range(G):
                    vector.tensor_mul(
                        out=_ap(prod, g * D * T * D,
                                [(T * D, D), (D, T), (1, D)]),
                        in0=_ap(o2b, g * T * D, [(0, D), (D, T), (1, D)]),
                        in1=_ap(wsh, OFFH_WO, [(D, D), (0, T), (1, D)]))
                vector.tensor_reduce(
                    out=_ap(res, s * XIN_SET, [(T * D, G), (1, D), (D, T)]),
                    in_=_ap(prod, 0, [(D, G * D * T), (1, D)]),
                    axis=mybir.AxisListType.X, op=mybir.AluOpType.add
                ).then_inc(sems["res_done"], 1)

            for n in range(NT):
                phase_a(n)
                if n >= 1:
                    phase_b(n - 1)
            phase_b(NT - 1)

    return nc


def _pack_weights(Wq, Wk, Wv, Wo):
    wts = np.zeros(CLEN, dtype=np.float32)
    scale = 1.0 / math.sqrt(HD)
    A2 = wts[OFF_A2:OFF_A2 + 36].reshape(2, D, POS)
    for h in range(NH):
        A2[0, h * HD:(h + 1) * HD, :] = (Wq[h * HD:(h + 1) * HD, :].T
                                         @ Wk[h * HD:(h + 1) * HD, :]) * scale
        A2[1, h * HD:(h + 1) * HD, :] = Wv[h * HD:(h + 1) * HD, :]
    wth = np.zeros(HLEN, dtype=np.float16)
    mask = np.where(np.tril(np.ones((T, T))) > 0, 0.0, MASKV).astype(np.float16)
    wth[OFFH_MASK:OFFH_MASK + TT] = mask.reshape(-1)
    wth[OFFH_WO:OFFH_WO + 36] = Wo.reshape(-1).astype(np.float16)
    return wts, wth


@lru_cache(maxsize=2)
def _cached_kernel(bc, G):
    return build_kernel(bc, G)


def kernel(x, Wq, Wk, Wv, Wo):
    x = np.ascontiguousarray(x, dtype=np.float32)
    B = x.shape[0]
    bc = B // NCORES
    G = 2
    nc = _cached_kernel(bc, G)
    wts, wth = _pack_weights(np.asarray(Wq, dtype=np.float32),
                             np.asarray(Wk, dtype=np.float32),
                             np.asarray(Wv, dtype=np.float32),
                             np.asarray(Wo, dtype=np.float32))
    in_maps = [{"x": x[i * bc:(i + 1) * bc], "wts": wts, "wth": wth}
               for i in range(NCORES)]
    r = run_bass_kernel_spmd(nc, in_maps, core_ids=list(range(NCORES)))
    return np.concatenate([m["out"] for m in r.results], axis=0)


# revision 10
# speedup vs baseline: 1.5542x; 1.5542x over previous
"""Trainium2 Bass kernel for nn_Attn_40046275068166.

Tiny causal MHA over huge batch: x[B=65536, T=34, D=6], 2 heads, head_dim 3.
Pure data parallelism over 8 cores; batch on the 128 SBUF partitions inside
each core; per-example compute in the free dims.

v2 design notes (calibrated on HW probes):
- DVE tensor_tensor runs 2x ONLY when every operand is 2-byte, innermost
  stride +-1, 4B-aligned start, EVEN run length. Odd 17-runs degrade to
  ~0.8 ns/elem. So score planes are padded to J=36 columns and split at
  j=18; all fp16 ops use even runs.
- tensor_reduce is always 1x (~1.05 ns/elem) -> fold rows with 2x fp16
  adds (36->18->(8+2)) before reducing.
- GPSIMD tensor ops share SBUF ports with DVE and starve it 4.5x ->
  gpsimd does DMA orchestration ONLY.
- ACT (scalar engine) runs ~0.87 ns/elem on any strides; it materializes
  the xp broadcast planes (xpb) and the x transpose (xt16) so every DVE
  mul is all-fp16 stride-1, and computes exp.
- Weights are compile-time -> folded q/k projection A = Wq^T Wk / sqrt(hd);
  mask additive -30000 (fp16-safe, exp -> 0) covers the causal upper
  triangles AND the j=34,35 pad columns.

Pipeline: Pool streams x in / res out (SWDGE, parity-split semaphores);
ACT preps tile n+1 (xt16/xpb) then exps tile n; DVE does phase_a(n)
(proj+scores) then phase_b(n-1) (den/PV/out-projection).
"""

import math
from contextlib import ExitStack
from functools import lru_cache

import numpy as np

import concourse.bass as bass
from concourse import mybir
from concourse.bass_utils import run_bass_kernel_spmd

NCORES = 8
T = 34
J = 36              # padded score-plane width
H = 17              # i split
JH = 18             # j split
D = 6
NH = 2
HD = 3
POS = 3
P = 128

F32 = mybir.dt.float32
F16 = mybir.dt.float16

# fp16 constants layout (element offsets)
OFFH_A2 = 0                 # [2][6][3]
OFFH_MA = 36                # maskA [17][18]
OFFH_MC = 36 + H * JH       # maskC [17][18]
OFFH_WO = 36 + 2 * H * JH   # WoM replicated [6][34][6] (WoM[dm][t][e]=Wo[dm][e])
HLEN = OFFH_WO + D * T * D

MASKV = -30000.0


def _ap(t, off, dims):
    p0 = t[:].ap[0]
    return bass.AP(tensor=t, offset=off, ap=[list(p0)] + [list(d) for d in dims])


def build_kernel(bc, G):
    assert bc % (P * G) == 0
    NT = bc // (P * G)

    nc = bass.Bass("TRN2")
    x = nc.dram_tensor("x", [bc, T, D], F32, kind="ExternalInput")
    wth = nc.dram_tensor("wth", [HLEN], F16, kind="ExternalInput")
    out = nc.dram_tensor("out", [bc, T, D], F32, kind="ExternalOutput")

    xr = x[:].rearrange("(n g p) t d -> n p g t d", g=G, p=P)
    outr = out[:].rearrange("(n g p) t d -> n p g t d", g=G, p=P)
    wth_b = bass.AP(tensor=wth, offset=0, ap=[[0, P], [1, HLEN]])

    # element strides
    XIN_SET = G * T * D         # 408
    XIN_G = T * D               # 34*6
    XT_SET = G * D * T          # xt16 [set][g][d][t]
    XT_G = D * T
    XPB_SET = POS * G * T * J   # xpb [set][a][g][i][j]
    XPB_A = G * T * J
    XPB_G = T * J
    YV_SET = 2 * G * D * J      # yv [set][w][g][hc][j]
    YV_W = G * D * J
    YV_G = D * J
    TS_SET = G * NH * T * J     # t0/t1 [set][g][h][i][j]
    TS_G = NH * T * J
    TS_H = T * J
    PP_G = NH * T * J           # pp [g][h][i][j]
    PP_H = T * J
    ED_G = NH * H * JH          # ed [g][h][i'][j']
    ED_H = H * JH
    DEN_G = NH * T
    O2_G = T * D
    PR_G = D * T * D            # prod [g][dm][t][e]
    RES_SET = G * T * D

    with ExitStack() as ctx:
        sb = lambda nm, shape, dt=F32: ctx.enter_context(
            nc.sbuf_tensor(nm, shape, dt))
        wsh = sb("wsh", [P, HLEN], F16)
        xin = sb("xin", [P, 2, G, T, D])
        xt16 = sb("xt16", [P, 2, G, D, T], F16)
        xpb = sb("xpb", [P, 2, POS, G, T, J], F16)
        yv = sb("yv", [P, 2, 2, G, D, J], F16)
        t0 = sb("t0", [P, 2, G, NH, T, J], F16)
        t1 = sb("t1", [P, 2, G, NH, T, J], F16)
        tmp = sb("tmp", [P, G, D, J], F16)
        ed = sb("ed", [P, G, NH, H, JH], F16)
        pp = sb("pp", [P, G, NH, T, J], F16)
        den = sb("den", [P, G, NH, T])
        rcp = sb("rcp", [P, G, NH, T])
        o2 = sb("o2", [P, G, T, D])
        o2b = sb("o2b", [P, G, T, D], F16)
        prod = sb("prod", [P, G, D, T, D], F16)
        res = sb("res", [P, 2, G, T, D])

        sem_names = ["dma_in0", "dma_in1", "const", "xin_done", "prep_done",
                     "s_done", "e_done", "b_done", "res_done", "out0", "out1"]
        sems = {k: ctx.enter_context(nc.semaphore(name=k)) for k in sem_names}

        block = ctx.enter_context(nc.Block())

        @block.gpsimd
        def _(sync):
            # DMA orchestration ONLY (gpsimd tensor ops would starve DVE's
            # SBUF ports). SWDGE: one +16 completion inc per dma_start.
            def store(k):
                sp = k % 2
                sync.wait_ge(sems["res_done"], k + 1)
                sync.dma_start(
                    out=outr[k],
                    in_=_ap(res, sp * RES_SET, [(XIN_G, G), (1, T * D)]),
                ).then_inc(sems["out0" if sp == 0 else "out1"], 16)

            sync.dma_start(out=wsh[:], in_=wth_b).then_inc(sems["const"], 16)
            for n in range(NT):
                s = n % 2
                if n >= 2:
                    sync.wait_ge(sems["xin_done"], n - 1)
                sync.dma_start(
                    out=_ap(xin, s * XIN_SET, [(XIN_G, G), (1, T * D)]),
                    in_=xr[n],
                ).then_inc(sems["dma_in0" if s == 0 else "dma_in1"], 16)
                if n >= 2:
                    store(n - 2)
            store(NT - 2)
            store(NT - 1)
            sync.wait_ge(sems["out0"], 16 * ((NT + 1) // 2))
            sync.wait_ge(sems["out1"], 16 * (NT // 2))

        @block.scalar
        def _(scalar):
            Copy = mybir.ActivationFunctionType.Copy
            Exp = mybir.ActivationFunctionType.Exp

            def prep(n):
                s = n % 2
                scalar.wait_ge(sems["dma_in0" if s == 0 else "dma_in1"],
                               16 * (n // 2 + 1))
                # xt16[g][d][t] = x[g][t][d]
                scalar.activation(
                    out=_ap(xt16, s * XT_SET, [(XT_G, G), (T, D), (1, T)]),
                    in_=_ap(xin, s * XIN_SET, [(XIN_G, G), (1, D), (D, T)]),
                    func=Copy)
                # xpb[a][g][i][j] = xp[g][i][a]  (AB: all i, j<18; C: i,j>=split)
                for a in range(POS):
                    scalar.activation(
                        out=_ap(xpb, s * XPB_SET + a * XPB_A,
                                [(XPB_G, G), (J, T), (1, JH)]),
                        in_=_ap(xin, s * XIN_SET + POS + a,
                                [(XIN_G, G), (D, T), (0, JH)]),
                        func=Copy)
                    act = scalar.activation(
                        out=_ap(xpb, s * XPB_SET + a * XPB_A + H * J + JH,
                                [(XPB_G, G), (J, H), (1, JH)]),
                        in_=_ap(xin, s * XIN_SET + POS + a + H * D,
                                [(XIN_G, G), (D, H), (0, JH)]),
                        func=Copy)
                    if a == POS - 1:
                        act.then_inc(sems["xin_done"], 1)

            def expf(n):
                s = n % 2
                scalar.wait_ge(sems["s_done"], n + 1)
                if n >= 2:
                    scalar.wait_ge(sems["b_done"], n - 1)
                # AB: all i, j<18 ((g,h) merged: TS_G == NH*TS_H)
                scalar.activation(
                    out=_ap(t1, s * TS_SET, [(TS_H, G * NH), (J, T), (1, JH)]),
                    in_=_ap(t0, s * TS_SET, [(TS_H, G * NH), (J, T), (1, JH)]),
                    func=Exp)
                # C: i>=17, j>=18
                scalar.activation(
                    out=_ap(t1, s * TS_SET + H * J + JH,
                            [(TS_H, G * NH), (J, H), (1, JH)]),
                    in_=_ap(t0, s * TS_SET + H * J + JH,
                            [(TS_H, G * NH), (J, H), (1, JH)]),
                    func=Exp,
                ).then_inc(sems["e_done"], 1)

            scalar.wait_ge(sems["const"], 16)
            prep(0)
            prep(1)
            for n in range(NT):
                expf(n)
                if n + 2 < NT:
                    prep(n + 2)

        @block.vector
        def _(vector):
            vector.wait_ge(sems["const"], 16)
            # zero the j=34,35 pad columns of yv once (proj never writes
            # them; PV/scores read them; mask kills their contribution but
            # they must be finite)
            vector.memset(
                _ap(yv, T, [(J, 2 * 2 * G * D), (1, J - T)]), 0.0)

            def phase_a(n):
                s = n % 2
                # xin_done is incremented by ACT prep(n)'s last copy, which
                # also certifies xt16/xpb for this tile
                vector.wait_ge(sems["xin_done"], n + 1)
                # projections: yv[w][g][hc][j] = sum_b xt16[g][3(1-w)+b][j] * A2[w][hc][b]
                for w in range(2):
                    for b in range(POS):
                        i0 = _ap(xt16, s * XT_SET + (3 * (1 - w) + b) * T,
                                 [(XT_G, G), (0, D), (1, T)])
                        i1 = _ap(wsh, OFFH_A2 + w * 18 + b,
                                 [(0, G), (3, D), (0, T)])
                        if b == 0:
                            vector.tensor_mul(
                                out=_ap(yv, s * YV_SET + w * YV_W,
                                        [(YV_G, G), (J, D), (1, T)]),
                                in0=i0, in1=i1)
                        else:
                            vector.tensor_mul(
                                out=_ap(tmp, 0, [(D * J, G), (J, D), (1, T)]),
                                in0=i0, in1=i1)
                            vector.tensor_add(
                                out=_ap(yv, s * YV_SET + w * YV_W,
                                        [(YV_G, G), (J, D), (1, T)]),
                                in0=_ap(yv, s * YV_SET + w * YV_W,
                                        [(YV_G, G), (J, D), (1, T)]),
                                in1=_ap(tmp, 0, [(D * J, G), (J, D), (1, T)]))
                # scores (smul split per head: xpb has no h dim to merge)
                def smul(a, dst):
                    for h in range(NH):
                        # AB: all i, j<18
                        vector.tensor_mul(
                            out=_ap(dst, s * TS_SET + h * TS_H,
                                    [(TS_G, G), (J, T), (1, JH)]),
                            in0=_ap(xpb, s * XPB_SET + a * XPB_A,
                                    [(XPB_G, G), (J, T), (1, JH)]),
                            in1=_ap(yv, s * YV_SET + (h * HD + a) * J,
                                    [(YV_G, G), (0, T), (1, JH)]))
                        # C: i>=17, j>=18
                        vector.tensor_mul(
                            out=_ap(dst, s * TS_SET + h * TS_H + H * J + JH,
                                    [(TS_G, G), (J, H), (1, JH)]),
                            in0=_ap(xpb, s * XPB_SET + a * XPB_A + H * J + JH,
                                    [(XPB_G, G), (J, H), (1, JH)]),
                            in1=_ap(yv, s * YV_SET + (h * HD + a) * J + JH,
                                    [(YV_G, G), (0, H), (1, JH)]))

                def tadd(region):
                    li = T if region == "AB" else H
                    off = 0 if region == "AB" else H * J + JH
                    vector.tensor_add(
                        out=_ap(t0, s * TS_SET + off,
                                [(TS_H, G * NH), (J, li), (1, JH)]),
                        in0=_ap(t0, s * TS_SET + off,
                                [(TS_H, G * NH), (J, li), (1, JH)]),
                        in1=_ap(t1, s * TS_SET + off,
                                [(TS_H, G * NH), (J, li), (1, JH)]))

                smul(0, t0)
                smul(1, t1)
                tadd("AB")
                tadd("C")
                smul(2, t1)
                # mask: t1[i<17, j<18] += maskA ; t1[C] += maskC
                vector.tensor_add(
                    out=_ap(t1, s * TS_SET, [(TS_H, G * NH), (J, H), (1, JH)]),
                    in0=_ap(t1, s * TS_SET, [(TS_H, G * NH), (J, H), (1, JH)]),
                    in1=_ap(wsh, OFFH_MA, [(0, G * NH), (JH, H), (1, JH)]))
                vector.tensor_add(
                    out=_ap(t1, s * TS_SET + H * J + JH,
                            [(TS_H, G * NH), (J, H), (1, JH)]),
                    in0=_ap(t1, s * TS_SET + H * J + JH,
                            [(TS_H, G * NH), (J, H), (1, JH)]),
                    in1=_ap(wsh, OFFH_MC, [(0, G * NH), (JH, H), (1, JH)]))
                tadd("AB")
                vector.tensor_add(
                    out=_ap(t0, s * TS_SET + H * J + JH,
                            [(TS_H, G * NH), (J, H), (1, JH)]),
                    in0=_ap(t0, s * TS_SET + H * J + JH,
                            [(TS_H, G * NH), (J, H), (1, JH)]),
                    in1=_ap(t1, s * TS_SET + H * J + JH,
                            [(TS_H, G * NH), (J, H), (1, JH)])
                ).then_inc(sems["s_done"], 1)

            def fold_chain(tsrc, soff, rows, into_ed):
                """rows i>=17: fold j[18:36) into j[0:18) (into ed or in
                place), then [16:18)->[0:2), [8:16)->[0:8)."""
                pass  # structured inline below for clarity

            def phase_b(n):
                s = n % 2
                vector.wait_ge(sems["e_done"], n + 1)
                if n >= 2:
                    vector.wait_ge(sems["out0" if s == 0 else "out1"],
                                   16 * (n // 2))
                # den: ed = e[i>=17, j<18] + e[i>=17, j>=18]  ((g,h) merged)
                vector.tensor_add(
                    out=_ap(ed, 0, [(ED_H, G * NH), (JH, H), (1, JH)]),
                    in0=_ap(t1, s * TS_SET + H * J,
                            [(TS_H, G * NH), (J, H), (1, JH)]),
                    in1=_ap(t1, s * TS_SET + H * J + JH,
                            [(TS_H, G * NH), (J, H), (1, JH)]))
                vector.tensor_add(
                    out=_ap(ed, 0, [(ED_H, G * NH), (JH, H), (1, 2)]),
                    in0=_ap(ed, 0, [(ED_H, G * NH), (JH, H), (1, 2)]),
                    in1=_ap(ed, 16, [(ED_H, G * NH), (JH, H), (1, 2)]))
                vector.tensor_add(
                    out=_ap(ed, 0, [(ED_H, G * NH), (JH, H), (1, 8)]),
                    in0=_ap(ed, 0, [(ED_H, G * NH), (JH, H), (1, 8)]),
                    in1=_ap(ed, 8, [(ED_H, G * NH), (JH, H), (1, 8)]))
                vector.tensor_reduce(
                    out=_ap(den, 0, [(T, G * NH), (1, H)]),
                    in_=_ap(t1, s * TS_SET,
                            [(TS_H, G * NH), (J, H), (1, JH)]),
                    axis=mybir.AxisListType.X, op=mybir.AluOpType.add)
                vector.tensor_reduce(
                    out=_ap(den, H, [(T, G * NH), (1, H)]),
                    in_=_ap(ed, 0, [(ED_H, G * NH), (JH, H), (1, 8)]),
                    axis=mybir.AxisListType.X, op=mybir.AluOpType.add)
                vector.reciprocal(
                    out=_ap(rcp, 0, [(1, G * NH * T)]),
                    in_=_ap(den, 0, [(1, G * NH * T)]))
                # PV per channel c  (yv (g,h) merge: YV_G == NH*HD*J)
                for c in range(HD):
                    vector.tensor_mul(
                        out=_ap(pp, 0, [(PP_H, G * NH), (J, T), (1, JH)]),
                        in0=_ap(t1, s * TS_SET,
                                [(TS_H, G * NH), (J, T), (1, JH)]),
                        in1=_ap(yv, s * YV_SET + YV_W + c * J,
                                [(HD * J, G * NH), (0, T), (1, JH)]))
                    mm = vector.tensor_mul(
                        out=_ap(pp, H * J + JH,
                                [(PP_H, G * NH), (J, H), (1, JH)]),
                        in0=_ap(t1, s * TS_SET + H * J + JH,
                                [(TS_H, G * NH), (J, H), (1, JH)]),
                        in1=_ap(yv, s * YV_SET + YV_W + c * J + JH,
                                [(HD * J, G * NH), (0, H), (1, JH)]))
                    if c == HD - 1:
                        mm.then_inc(sems["b_done"], 1)
                    vector.tensor_add(
                        out=_ap(pp, H * J, [(PP_H, G * NH), (J, H), (1, JH)]),
                        in0=_ap(pp, H * J, [(PP_H, G * NH), (J, H), (1, JH)]),
                        in1=_ap(pp, H * J + JH,
                                [(PP_H, G * NH), (J, H), (1, JH)]))
                    vector.tensor_add(
                        out=_ap(pp, 0, [(PP_H, G * NH), (J, T), (1, 2)]),
                        in0=_ap(pp, 0, [(PP_H, G * NH), (J, T), (1, 2)]),
                        in1=_ap(pp, 16, [(PP_H, G * NH), (J, T), (1, 2)]))
                    vector.tensor_add(
                        out=_ap(pp, 0, [(PP_H, G * NH), (J, T), (1, 8)]),
                        in0=_ap(pp, 0, [(PP_H, G * NH), (J, T), (1, 8)]),
                        in1=_ap(pp, 8, [(PP_H, G * NH), (J, T), (1, 8)]))
                    vector.tensor_reduce(
                        out=_ap(o2, c, [(O2_G, G), (HD, NH), (D, T)]),
                        in_=_ap(pp, 0, [(PP_H, G * NH), (J, T), (1, 8)]),
                        axis=mybir.AxisListType.X, op=mybir.AluOpType.add)
                # normalize -> fp16 (split per head: rcp not (h,c)-mergeable)
                for h in range(NH):
                    vector.tensor_mul(
                        out=_ap(o2b, h * HD,
                                [(O2_G, G), (D, T), (1, HD)]),
                        in0=_ap(o2, h * HD,
                                [(O2_G, G), (D, T), (1, HD)]),
                        in1=_ap(rcp, h * T,
                                [(DEN_G, G), (1, T), (0, HD)]))
                # output projection (WoM replicated over t -> (t,e) merge)
                vector.tensor_mul(
                    out=_ap(prod, 0, [(PR_G, G), (T * D, D), (1, T * D)]),
                    in0=_ap(o2b, 0, [(O2_G, G), (0, D), (1, T * D)]),
                    in1=_ap(wsh, OFFH_WO, [(0, G), (T * D, D), (1, T * D)]))
                vector.tensor_reduce(
                    out=_ap(res, s * RES_SET, [(RES_SET // G, G), (1, D), (D, T)]),
                    in_=_ap(prod, 0, [(PR_G, G), (D, D * T), (1, D)]),
                    axis=mybir.AxisListType.X, op=mybir.AluOpType.add
                ).then_inc(sems["res_done"], 1)

            for n in range(NT):
                phase_a(n)
                if n >= 1:
                    phase_b(n - 1)
            phase_b(NT - 1)

    return nc


def _pack_weights(Wq, Wk, Wv, Wo):
    wth = np.zeros(HLEN, dtype=np.float16)
    scale = 1.0 / math.sqrt(HD)
    A2 = np.zeros((2, D, POS), dtype=np.float64)
    for h in range(NH):
        A2[0, h * HD:(h + 1) * HD, :] = (Wq[h * HD:(h + 1) * HD, :].T
                                         @ Wk[h * HD:(h + 1) * HD, :]) * scale
        A2[1, h * HD:(h + 1) * HD, :] = Wv[h * HD:(h + 1) * HD, :]
    wth[OFFH_A2:OFFH_A2 + 36] = A2.reshape(-1).astype(np.float16)
    # WoM replicated over t: [dm][t][e] = Wo[dm][e]
    wom = np.broadcast_to(Wo.astype(np.float16)[:, None, :], (D, T, D))
    wth[OFFH_WO:OFFH_WO + D * T * D] = wom.reshape(-1)
    # maskA: rows i<17, cols j<18: -30000 where j > i
    ma = np.zeros((H, JH), dtype=np.float16)
    for i in range(H):
        ma[i, i + 1:] = MASKV
    # maskC: rows i'=i-17, cols j'=j-18: -30000 where j' >= i'  (covers pad)
    mc = np.zeros((H, JH), dtype=np.float16)
    for i in range(H):
        mc[i, i:] = MASKV
    wth[OFFH_MA:OFFH_MA + H * JH] = ma.reshape(-1)
    wth[OFFH_MC:OFFH_MC + H * JH] = mc.reshape(-1)
    return wth


@lru_cache(maxsize=2)
def _cached_kernel(bc, G):
    return build_kernel(bc, G)


def _prepare(x, Wq, Wk, Wv, Wo, G=2):
    x = np.ascontiguousarray(x, dtype=np.float32)
    B = x.shape[0]
    bc = B // NCORES
    nc = _cached_kernel(bc, G)
    wth = _pack_weights(np.asarray(Wq, dtype=np.float32),
                        np.asarray(Wk, dtype=np.float32),
                        np.asarray(Wv, dtype=np.float32),
                        np.asarray(Wo, dtype=np.float32))
    in_maps = [{"x": x[i * bc:(i + 1) * bc], "wth": wth}
               for i in range(NCORES)]
    return nc, in_maps


def kernel(x, Wq, Wk, Wv, Wo):
    nc, in_maps = _prepare(x, Wq, Wk, Wv, Wo)
    r = run_bass_kernel_spmd(nc, in_maps, core_ids=list(range(NCORES)))
    return np.concatenate([m["out"] for m in r.results], axis=0)


# revision 15
# speedup vs baseline: 1.6053x; 1.0329x over previous
"""Trainium2 Bass kernel for nn_Attn_40046275068166.

Tiny causal MHA over huge batch: x[B=65536, T=34, D=6], 2 heads, head_dim 3.
Pure data parallelism over 8 cores; batch on the 128 SBUF partitions inside
each core; per-example compute in the free dims.

v2 design notes (calibrated on HW probes):
- DVE tensor_tensor runs 2x ONLY when every operand is 2-byte, innermost
  stride +-1, 4B-aligned start, EVEN run length. Odd 17-runs degrade to
  ~0.8 ns/elem. So score planes are padded to J=36 columns and split at
  j=18; all fp16 ops use even runs.
- tensor_reduce is always 1x (~1.05 ns/elem) -> fold rows with 2x fp16
  adds (36->18->(8+2)) before reducing.
- GPSIMD tensor ops share SBUF ports with DVE and starve it 4.5x ->
  gpsimd does DMA orchestration ONLY.
- ACT (scalar engine) runs ~0.87 ns/elem on any strides; it materializes
  the xp broadcast planes (xpb) and the x transpose (xt16) so every DVE
  mul is all-fp16 stride-1, and computes exp.
- Weights are compile-time -> folded q/k projection A = Wq^T Wk / sqrt(hd);
  mask additive -30000 (fp16-safe, exp -> 0) covers the causal upper
  triangles AND the j=34,35 pad columns.

Pipeline: Pool streams x in / res out (SWDGE, parity-split semaphores);
ACT preps tile n+1 (xt16/xpb) then exps tile n; DVE does phase_a(n)
(proj+scores) then phase_b(n-1) (den/PV/out-projection).
"""

import math
from contextlib import ExitStack
from functools import lru_cache

import numpy as np

import concourse.bass as bass
from concourse import mybir
from concourse.bass_utils import run_bass_kernel_spmd

NCORES = 8
T = 34
J = 36              # padded score-plane width
H = 17              # i split
JH = 18             # j split
D = 6
NH = 2
HD = 3
POS = 3
P = 128

F32 = mybir.dt.float32
F16 = mybir.dt.float16

# fp16 constants layout (element offsets)
OFFH_MA = 0                 # maskA [17][18]
OFFH_MC = H * JH            # maskC [17][18]
OFFH_WO = 2 * H * JH        # WoM replicated [6][34][6] (WoM[dm][t][e]=Wo[dm][e])
OFFH_A2 = OFFH_WO + D * T * D   # A2 replicated [2][3(b)][6(hc)][34(t)]
HLEN = OFFH_A2 + 2 * POS * D * T

MASKV = -30000.0


def _ap(t, off, dims):
    p0 = t[:].ap[0]
    return bass.AP(tensor=t, offset=off, ap=[list(p0)] + [list(d) for d in dims])


def build_kernel(bc, G):
    assert bc % (P * G) == 0
    NT = bc // (P * G)

    nc = bass.Bass("TRN2")
    x = nc.dram_tensor("x", [bc, T, D], F32, kind="ExternalInput")
    wth = nc.dram_tensor("wth", [HLEN], F16, kind="ExternalInput")
    out = nc.dram_tensor("out", [bc, T, D], F32, kind="ExternalOutput")

    xr = x[:].rearrange("(n g p) t d -> n p g t d", g=G, p=P)
    outr = out[:].rearrange("(n g p) t d -> n p g t d", g=G, p=P)
    wth_b = bass.AP(tensor=wth, offset=0, ap=[[0, P], [1, HLEN]])

    # element strides
    XIN_SET = G * T * D         # 408
    XIN_G = T * D               # 34*6
    XT_SET = G * D * T          # xt16 [set][g][d][t]
    XT_G = D * T
    XPB_SET = POS * G * T * J   # xpb [set][a][g][i][j]
    XPB_A = G * T * J
    XPB_G = T * J
    YV_SET = 2 * G * D * J      # yv [set][w][g][hc][j]
    YV_W = G * D * J
    YV_G = D * J
    TS_SET = G * NH * T * J     # t0/t1 [set][g][h][i][j]
    TS_G = NH * T * J
    TS_H = T * J
    PP_G = NH * T * J           # pp [g][h][i][j]
    PP_H = T * J
    ED_G = NH * H * JH          # ed [g][h][i'][j']
    ED_H = H * JH
    DEN_G = NH * T
    O2_G = T * D
    PR_G = D * T * D            # prod [g][dm][t][e]
    RES_SET = G * T * D

    with ExitStack() as ctx:
        sb = lambda nm, shape, dt=F32: ctx.enter_context(
            nc.sbuf_tensor(nm, shape, dt))
        wsh = sb("wsh", [P, HLEN], F16)
        xin = sb("xin", [P, 2, G, T, D])
        xt16 = sb("xt16", [P, 2, G, D, T], F16)
        xpb = sb("xpb", [P, 2, POS, G, T, J], F16)
        yv = sb("yv", [P, 2, 2, G, D, J], F16)
        t0 = sb("t0", [P, 2, G, NH, T, J], F16)
        t1 = sb("t1", [P, 2, G, NH, T, J], F16)
        tmp = sb("tmp", [P, G, D, J], F16)
        ed = sb("ed", [P, G, NH, H, JH], F16)
        pp = sb("pp", [P, G, NH, T, J], F16)
        den = sb("den", [P, G, NH, T])
        rcp = sb("rcp", [P, G, NH, T])
        o2 = sb("o2", [P, G, T, D])
        o2b = sb("o2b", [P, G, T, D], F16)
        prod = sb("prod", [P, G, D, T, D], F16)
        res = sb("res", [P, 2, G, T, D])

        sem_names = ["dma_in0", "dma_in1", "const", "xin_done", "prep_done",
                     "s_done", "e_done", "b_done", "res_done", "out0", "out1"]
        sems = {k: ctx.enter_context(nc.semaphore(name=k)) for k in sem_names}

        block = ctx.enter_context(nc.Block())

        @block.gpsimd
        def _(sync):
            # DMA orchestration ONLY (gpsimd tensor ops would starve DVE's
            # SBUF ports). SWDGE: one +16 completion inc per dma_start.
            def store(k):
                sp = k % 2
                sync.wait_ge(sems["res_done"], k + 1)
                sync.dma_start(
                    out=outr[k],
                    in_=_ap(res, sp * RES_SET, [(XIN_G, G), (1, T * D)]),
                ).then_inc(sems["out0" if sp == 0 else "out1"], 16)

            sync.dma_start(out=wsh[:], in_=wth_b).then_inc(sems["const"], 16)
            for n in range(NT):
                s = n % 2
                if n >= 2:
                    sync.wait_ge(sems["xin_done"], n - 1)
                sync.dma_start(
                    out=_ap(xin, s * XIN_SET, [(XIN_G, G), (1, T * D)]),
                    in_=xr[n],
                ).then_inc(sems["dma_in0" if s == 0 else "dma_in1"], 16)
                if n >= 2:
                    store(n - 2)
            store(NT - 2)
            store(NT - 1)
            sync.wait_ge(sems["out0"], 16 * ((NT + 1) // 2))
            sync.wait_ge(sems["out1"], 16 * (NT // 2))

        @block.scalar
        def _(scalar):
            Copy = mybir.ActivationFunctionType.Copy
            Exp = mybir.ActivationFunctionType.Exp

            def prep(n):
                s = n % 2
                scalar.wait_ge(sems["dma_in0" if s == 0 else "dma_in1"],
                               16 * (n // 2 + 1))
                # xt16[g][d][t] = x[g][t][d]
                scalar.activation(
                    out=_ap(xt16, s * XT_SET, [(XT_G, G), (T, D), (1, T)]),
                    in_=_ap(xin, s * XIN_SET, [(XIN_G, G), (1, D), (D, T)]),
                    func=Copy)
                # xpb[a][g][i][j] = xp[g][i][a]  (AB: all i, j<18; C: i,j>=split)
                for a in range(POS):
                    scalar.activation(
                        out=_ap(xpb, s * XPB_SET + a * XPB_A,
                                [(XPB_G, G), (J, T), (1, JH)]),
                        in_=_ap(xin, s * XIN_SET + POS + a,
                                [(XIN_G, G), (D, T), (0, JH)]),
                        func=Copy)
                    act = scalar.activation(
                        out=_ap(xpb, s * XPB_SET + a * XPB_A + H * J + JH,
                                [(XPB_G, G), (J, H), (1, JH)]),
                        in_=_ap(xin, s * XIN_SET + POS + a + H * D,
                                [(XIN_G, G), (D, H), (0, JH)]),
                        func=Copy)
                    if a == POS - 1:
                        act.then_inc(sems["xin_done"], 1)

            def expf(n):
                s = n % 2
                scalar.wait_ge(sems["s_done"], n + 1)
                if n >= 2:
                    scalar.wait_ge(sems["b_done"], n - 1)
                # AB: all i, j<18 ((g,h) merged: TS_G == NH*TS_H)
                scalar.activation(
                    out=_ap(t1, s * TS_SET, [(TS_H, G * NH), (J, T), (1, JH)]),
                    in_=_ap(t0, s * TS_SET, [(TS_H, G * NH), (J, T), (1, JH)]),
                    func=Exp)
                # C: i>=17, j>=18
                scalar.activation(
                    out=_ap(t1, s * TS_SET + H * J + JH,
                            [(TS_H, G * NH), (J, H), (1, JH)]),
                    in_=_ap(t0, s * TS_SET + H * J + JH,
                            [(TS_H, G * NH), (J, H), (1, JH)]),
                    func=Exp,
                ).then_inc(sems["e_done"], 1)

            scalar.wait_ge(sems["const"], 16)
            prep(0)
            prep(1)
            for n in range(NT):
                expf(n)
                if n + 2 < NT:
                    prep(n + 2)

        @block.vector
        def _(vector):
            vector.wait_ge(sems["const"], 16)
            # zero the j=34,35 pad columns of yv once (proj never writes
            # them; PV/scores read them; mask kills their contribution but
            # they must be finite)
            vector.memset(
                _ap(yv, T, [(J, 2 * 2 * G * D), (1, J - T)]), 0.0)

            def phase_a(n):
                s = n % 2
                # xin_done is incremented by ACT prep(n)'s last copy, which
                # also certifies xt16/xpb for this tile
                vector.wait_ge(sems["xin_done"], n + 1)
                # projections: yv[w][g][hc][j] = sum_b xt16[g][3(1-w)+b][j] * A2[w][hc][b]
                for w in range(2):
                    for b in range(POS):
                        i0 = _ap(xt16, s * XT_SET + (3 * (1 - w) + b) * T,
                                 [(XT_G, G), (0, D), (1, T)])
                        i1 = _ap(wsh, OFFH_A2 + (w * POS + b) * D * T,
                                 [(0, G), (T, D), (1, T)])
                        if b == 0:
                            vector.tensor_mul(
                                out=_ap(yv, s * YV_SET + w * YV_W,
                                        [(YV_G, G), (J, D), (1, T)]),
                                in0=i0, in1=i1)
                        else:
                            vector.tensor_mul(
                                out=_ap(tmp, 0, [(D * J, G), (J, D), (1, T)]),
                                in0=i0, in1=i1)
                            vector.tensor_add(
                                out=_ap(yv, s * YV_SET + w * YV_W,
                                        [(YV_G, G), (J, D), (1, T)]),
                                in0=_ap(yv, s * YV_SET + w * YV_W,
                                        [(YV_G, G), (J, D), (1, T)]),
                                in1=_ap(tmp, 0, [(D * J, G), (J, D), (1, T)]))
                # scores (smul split per head: xpb has no h dim to merge)
                def smul(a, dst):
                    for h in range(NH):
                        # AB: all i, j<18
                        vector.tensor_mul(
                            out=_ap(dst, s * TS_SET + h * TS_H,
                                    [(TS_G, G), (J, T), (1, JH)]),
                            in0=_ap(xpb, s * XPB_SET + a * XPB_A,
                                    [(XPB_G, G), (J, T), (1, JH)]),
                            in1=_ap(yv, s * YV_SET + (h * HD + a) * J,
                                    [(YV_G, G), (0, T), (1, JH)]))
                        # C: i>=17, j>=18
                        vector.tensor_mul(
                            out=_ap(dst, s * TS_SET + h * TS_H + H * J + JH,
                                    [(TS_G, G), (J, H), (1, JH)]),
                            in0=_ap(xpb, s * XPB_SET + a * XPB_A + H * J + JH,
                                    [(XPB_G, G), (J, H), (1, JH)]),
                            in1=_ap(yv, s * YV_SET + (h * HD + a) * J + JH,
                                    [(YV_G, G), (0, H), (1, JH)]))

                def tadd(region):
                    li = T if region == "AB" else H
                    off = 0 if region == "AB" else H * J + JH
                    vector.tensor_add(
                        out=_ap(t0, s * TS_SET + off,
                                [(TS_H, G * NH), (J, li), (1, JH)]),
                        in0=_ap(t0, s * TS_SET + off,
                                [(TS_H, G * NH), (J, li), (1, JH)]),
                        in1=_ap(t1, s * TS_SET + off,
                                [(TS_H, G * NH), (J, li), (1, JH)]))

                smul(0, t0)
                smul(1, t1)
                tadd("AB")
                tadd("C")
                smul(2, t1)
                # mask: t1[i<17, j<18] += maskA ; t1[C] += maskC
                vector.tensor_add(
                    out=_ap(t1, s * TS_SET, [(TS_H, G * NH), (J, H), (1, JH)]),
                    in0=_ap(t1, s * TS_SET, [(TS_H, G * NH), (J, H), (1, JH)]),
                    in1=_ap(wsh, OFFH_MA, [(0, G * NH), (JH, H), (1, JH)]))
                vector.tensor_add(
                    out=_ap(t1, s * TS_SET + H * J + JH,
                            [(TS_H, G * NH), (J, H), (1, JH)]),
                    in0=_ap(t1, s * TS_SET + H * J + JH,
                            [(TS_H, G * NH), (J, H), (1, JH)]),
                    in1=_ap(wsh, OFFH_MC, [(0, G * NH), (JH, H), (1, JH)]))
                tadd("AB")
                vector.tensor_add(
                    out=_ap(t0, s * TS_SET + H * J + JH,
                            [(TS_H, G * NH), (J, H), (1, JH)]),
                    in0=_ap(t0, s * TS_SET + H * J + JH,
                            [(TS_H, G * NH), (J, H), (1, JH)]),
                    in1=_ap(t1, s * TS_SET + H * J + JH,
                            [(TS_H, G * NH), (J, H), (1, JH)])
                ).then_inc(sems["s_done"], 1)

            def fold_chain(tsrc, soff, rows, into_ed):
                """rows i>=17: fold j[18:36) into j[0:18) (into ed or in
                place), then [16:18)->[0:2), [8:16)->[0:8)."""
                pass  # structured inline below for clarity

            def phase_b(n):
                s = n % 2
                vector.wait_ge(sems["e_done"], n + 1)
                if n >= 2:
                    vector.wait_ge(sems["out0" if s == 0 else "out1"],
                                   16 * (n // 2))
                # den: ed = e[i>=17, j<18] + e[i>=17, j>=18]  ((g,h) merged)
                vector.tensor_add(
                    out=_ap(ed, 0, [(ED_H, G * NH), (JH, H), (1, JH)]),
                    in0=_ap(t1, s * TS_SET + H * J,
                            [(TS_H, G * NH), (J, H), (1, JH)]),
                    in1=_ap(t1, s * TS_SET + H * J + JH,
                            [(TS_H, G * NH), (J, H), (1, JH)]))
                # fold [10:18) onto [0:8), then reduce the contiguous [0:10)
                vector.tensor_add(
                    out=_ap(ed, 0, [(ED_H, G * NH), (JH, H), (1, 8)]),
                    in0=_ap(ed, 0, [(ED_H, G * NH), (JH, H), (1, 8)]),
                    in1=_ap(ed, 10, [(ED_H, G * NH), (JH, H), (1, 8)]))
                vector.tensor_reduce(
                    out=_ap(den, 0, [(T, G * NH), (1, H)]),
                    in_=_ap(t1, s * TS_SET,
                            [(TS_H, G * NH), (J, H), (1, JH)]),
                    axis=mybir.AxisListType.X, op=mybir.AluOpType.add)
                vector.tensor_reduce(
                    out=_ap(den, H, [(T, G * NH), (1, H)]),
                    in_=_ap(ed, 0, [(ED_H, G * NH), (JH, H), (1, 10)]),
                    axis=mybir.AxisListType.X, op=mybir.AluOpType.add)
                vector.reciprocal(
                    out=_ap(rcp, 0, [(1, G * NH * T)]),
                    in_=_ap(den, 0, [(1, G * NH * T)]))
                # PV per channel c  (yv (g,h) merge: YV_G == NH*HD*J)
                for c in range(HD):
                    vector.tensor_mul(
                        out=_ap(pp, 0, [(PP_H, G * NH), (J, T), (1, JH)]),
                        in0=_ap(t1, s * TS_SET,
                                [(TS_H, G * NH), (J, T), (1, JH)]),
                        in1=_ap(yv, s * YV_SET + YV_W + c * J,
                                [(HD * J, G * NH), (0, T), (1, JH)]))
                    mm = vector.tensor_mul(
                        out=_ap(pp, H * J + JH,
                                [(PP_H, G * NH), (J, H), (1, JH)]),
                        in0=_ap(t1, s * TS_SET + H * J + JH,
                                [(TS_H, G * NH), (J, H), (1, JH)]),
                        in1=_ap(yv, s * YV_SET + YV_W + c * J + JH,
                                [(HD * J, G * NH), (0, H), (1, JH)]))
                    if c == HD - 1:
                        mm.then_inc(sems["b_done"], 1)
                    vector.tensor_add(
                        out=_ap(pp, H * J, [(PP_H, G * NH), (J, H), (1, JH)]),
                        in0=_ap(pp, H * J, [(PP_H, G * NH), (J, H), (1, JH)]),
                        in1=_ap(pp, H * J + JH,
                                [(PP_H, G * NH), (J, H), (1, JH)]))
                    vector.tensor_add(
                        out=_ap(pp, 0, [(PP_H, G * NH), (J, T), (1, 8)]),
                        in0=_ap(pp, 0, [(PP_H, G * NH), (J, T), (1, 8)]),
                        in1=_ap(pp, 10, [(PP_H, G * NH), (J, T), (1, 8)]))
                    vector.tensor_reduce(
                        out=_ap(o2, c, [(O2_G, G), (HD, NH), (D, T)]),
                        in_=_ap(pp, 0, [(PP_H, G * NH), (J, T), (1, 10)]),
                        axis=mybir.AxisListType.X, op=mybir.AluOpType.add)
                # normalize -> fp16 (split per head: rcp not (h,c)-mergeable)
                for h in range(NH):
                    vector.tensor_mul(
                        out=_ap(o2b, h * HD,
                                [(O2_G, G), (D, T), (1, HD)]),
                        in0=_ap(o2, h * HD,
                                [(O2_G, G), (D, T), (1, HD)]),
                        in1=_ap(rcp, h * T,
                                [(DEN_G, G), (1, T), (0, HD)]))
                # output projection (WoM replicated over t -> (t,e) merge)
                vector.tensor_mul(
                    out=_ap(prod, 0, [(PR_G, G), (T * D, D), (1, T * D)]),
                    in0=_ap(o2b, 0, [(O2_G, G), (0, D), (1, T * D)]),
                    in1=_ap(wsh, OFFH_WO, [(0, G), (T * D, D), (1, T * D)]))
                vector.tensor_reduce(
                    out=_ap(res, s * RES_SET, [(RES_SET // G, G), (1, D), (D, T)]),
                    in_=_ap(prod, 0, [(PR_G, G), (D, D * T), (1, D)]),
                    axis=mybir.AxisListType.X, op=mybir.AluOpType.add
                ).then_inc(sems["res_done"], 1)

            for n in range(NT):
                phase_a(n)
                if n >= 1:
                    phase_b(n - 1)
            phase_b(NT - 1)

    return nc


def _pack_weights(Wq, Wk, Wv, Wo):
    wth = np.zeros(HLEN, dtype=np.float16)
    scale = 1.0 / math.sqrt(HD)
    A2 = np.zeros((2, D, POS), dtype=np.float64)
    for h in range(NH):
        A2[0, h * HD:(h + 1) * HD, :] = (Wq[h * HD:(h + 1) * HD, :].T
                                         @ Wk[h * HD:(h + 1) * HD, :]) * scale
        A2[1, h * HD:(h + 1) * HD, :] = Wv[h * HD:(h + 1) * HD, :]
    # A2 replicated over t: [w][b][hc][t] = A2[w][hc][b]
    a2r = np.broadcast_to(
        A2.astype(np.float16).transpose(0, 2, 1)[:, :, :, None],
        (2, POS, D, T))
    wth[OFFH_A2:OFFH_A2 + 2 * POS * D * T] = a2r.reshape(-1)
    # WoM replicated over t: [dm][t][e] = Wo[dm][e]
    wom = np.broadcast_to(Wo.astype(np.float16)[:, None, :], (D, T, D))
    wth[OFFH_WO:OFFH_WO + D * T * D] = wom.reshape(-1)
    # maskA: rows i<17, cols j<18: -30000 where j > i
    ma = np.zeros((H, JH), dtype=np.float16)
    for i in range(H):
        ma[i, i + 1:] = MASKV
    # maskC: rows i'=i-17, cols j'=j-18: -30000 where j' >= i'  (covers pad)
    mc = np.zeros((H, JH), dtype=np.float16)
    for i in range(H):
        mc[i, i:] = MASKV
    wth[OFFH_MA:OFFH_MA + H * JH] = ma.reshape(-1)
    wth[OFFH_MC:OFFH_MC + H * JH] = mc.reshape(-1)
    return wth


@lru_cache(maxsize=2)
def _cached_kernel(bc, G):
    return build_kernel(bc, G)


def _prepare(x, Wq, Wk, Wv, Wo, G=2):
    x = np.ascontiguousarray(x, dtype=np.float32)
    B = x.shape[0]
    bc = B // NCORES
    nc = _cached_kernel(bc, G)
    wth = _pack_weights(np.asarray(Wq, dtype=np.float32),
                        np.asarray(Wk, dtype=np.float32),
                        np.asarray(Wv, dtype=np.float32),
                        np.asarray(Wo, dtype=np.float32))
    in_maps = [{"x": x[i * bc:(i + 1) * bc], "wth": wth}
               for i in range(NCORES)]
    return nc, in_maps


def kernel(x, Wq, Wk, Wv, Wo):
    nc, in_maps = _prepare(x, Wq, Wk, Wv, Wo)
    r = run_bass_kernel_spmd(nc, in_maps, core_ids=list(range(NCORES)))
    return np.concatenate([m["out"] for m in r.results], axis=0)


# revision 17
# speedup vs baseline: 1.6097x; 1.0028x over previous
"""Trainium2 Bass kernel for nn_Attn_40046275068166.

Tiny causal MHA over huge batch: x[B=65536, T=34, D=6], 2 heads, head_dim 3.
Pure data parallelism over 8 cores; batch on the 128 SBUF partitions inside
each core; per-example compute in the free dims.

v2 design notes (calibrated on HW probes):
- DVE tensor_tensor runs 2x ONLY when every operand is 2-byte, innermost
  stride +-1, 4B-aligned start, EVEN run length. Odd 17-runs degrade to
  ~0.8 ns/elem. So score planes are padded to J=36 columns and split at
  j=18; all fp16 ops use even runs.
- tensor_reduce is always 1x (~1.05 ns/elem) -> fold rows with 2x fp16
  adds (36->18->(8+2)) before reducing.
- GPSIMD tensor ops share SBUF ports with DVE and starve it 4.5x ->
  gpsimd does DMA orchestration ONLY.
- ACT (scalar engine) runs ~0.87 ns/elem on any strides; it materializes
  the xp broadcast planes (xpb) and the x transpose (xt16) so every DVE
  mul is all-fp16 stride-1, and computes exp.
- Weights are compile-time -> folded q/k projection A = Wq^T Wk / sqrt(hd);
  mask additive -30000 (fp16-safe, exp -> 0) covers the causal upper
  triangles AND the j=34,35 pad columns.

Pipeline: Pool streams x in / res out (SWDGE, parity-split semaphores);
ACT preps tile n+1 (xt16/xpb) then exps tile n; DVE does phase_a(n)
(proj+scores) then phase_b(n-1) (den/PV/out-projection).
"""

import math
from contextlib import ExitStack
from functools import lru_cache

import numpy as np

import concourse.bass as bass
from concourse import mybir
from concourse.bass_utils import run_bass_kernel_spmd

NCORES = 8
T = 34
J = 36              # padded score-plane width
H = 17              # i split
JH = 18             # j split
D = 6
NH = 2
HD = 3
POS = 3
P = 128

F32 = mybir.dt.float32
F16 = mybir.dt.float16

# fp16 constants layout (element offsets)
OFFH_MA = 0                 # maskA [17][18]
OFFH_MC = H * JH            # maskC [17][18]
OFFH_WO = 2 * H * JH        # WoM replicated [6][34][6] (WoM[dm][t][e]=Wo[dm][e])
OFFH_A2 = OFFH_WO + D * T * D   # A2 replicated [2][3(b)][6(hc)][34(t)]
HLEN = OFFH_A2 + 2 * POS * D * T

MASKV = -30000.0


def _ap(t, off, dims):
    p0 = t[:].ap[0]
    return bass.AP(tensor=t, offset=off, ap=[list(p0)] + [list(d) for d in dims])


def build_kernel(bc, G):
    assert bc % (P * G) == 0
    NT = bc // (P * G)

    nc = bass.Bass("TRN2")
    x = nc.dram_tensor("x", [bc, T, D], F32, kind="ExternalInput")
    wth = nc.dram_tensor("wth", [HLEN], F16, kind="ExternalInput")
    out = nc.dram_tensor("out", [bc, T, D], F32, kind="ExternalOutput")

    xr = x[:].rearrange("(n g p) t d -> n p g t d", g=G, p=P)
    outr = out[:].rearrange("(n g p) t d -> n p g t d", g=G, p=P)
    wth_b = bass.AP(tensor=wth, offset=0, ap=[[0, P], [1, HLEN]])

    # element strides
    XIN_SET = G * T * D         # 408
    XIN_G = T * D               # 34*6
    XT_SET = G * D * T          # xt16 [set][g][d][t]
    XT_G = D * T
    XPB_SET = POS * G * T * JH  # xpb [set][a][g][i][j<18] (C reuses cols 0:18)
    XPB_A = G * T * JH
    XPB_G = T * JH
    YV_SET = 2 * G * D * J      # yv [set][w][g][hc][j]
    YV_W = G * D * J
    YV_G = D * J
    TS_SET = G * NH * T * J     # t0/t1 [set][g][h][i][j]
    TS_G = NH * T * J
    TS_H = T * J
    PP_G = NH * T * J           # pp [g][h][i][j]
    PP_H = T * J
    ED_G = NH * H * JH          # ed [g][h][i'][j']
    ED_H = H * JH
    DEN_G = NH * T
    O2_G = T * D
    PR_G = D * T * D            # prod [g][dm][t][e]
    RES_SET = G * T * D

    with ExitStack() as ctx:
        sb = lambda nm, shape, dt=F32: ctx.enter_context(
            nc.sbuf_tensor(nm, shape, dt))
        wsh = sb("wsh", [P, HLEN], F16)
        xin = sb("xin", [P, 2, G, T, D])
        xt16 = sb("xt16", [P, 2, G, D, T], F16)
        xpb = sb("xpb", [P, 2, POS, G, T, JH], F16)
        yv = sb("yv", [P, 2, 2, G, D, J], F16)
        t0 = sb("t0", [P, 2, G, NH, T, J], F16)
        t1 = sb("t1", [P, 2, G, NH, T, J], F16)
        tmp = sb("tmp", [P, G, D, J], F16)
        ed = sb("ed", [P, G, NH, H, JH], F16)
        pp = sb("pp", [P, G, NH, T, J], F16)
        den = sb("den", [P, G, NH, T])
        rcp = sb("rcp", [P, G, NH, T])
        o2 = sb("o2", [P, G, T, D])
        o2b = sb("o2b", [P, G, T, D], F16)
        prod = sb("prod", [P, G, D, T, D], F16)
        res = sb("res", [P, 2, G, T, D])

        sem_names = ["dma_in0", "dma_in1", "const", "xin_done", "prep_done",
                     "s_done", "e_done", "b_done", "res_done", "out0", "out1"]
        sems = {k: ctx.enter_context(nc.semaphore(name=k)) for k in sem_names}

        block = ctx.enter_context(nc.Block())

        @block.gpsimd
        def _(sync):
            # DMA orchestration ONLY (gpsimd tensor ops would starve DVE's
            # SBUF ports). SWDGE: one +16 completion inc per dma_start.
            def store(k):
                sp = k % 2
                sync.wait_ge(sems["res_done"], k + 1)
                sync.dma_start(
                    out=outr[k],
                    in_=_ap(res, sp * RES_SET, [(XIN_G, G), (1, T * D)]),
                ).then_inc(sems["out0" if sp == 0 else "out1"], 16)

            sync.dma_start(out=wsh[:], in_=wth_b).then_inc(sems["const"], 16)
            for n in range(NT):
                s = n % 2
                if n >= 2:
                    sync.wait_ge(sems["xin_done"], n - 1)
                sync.dma_start(
                    out=_ap(xin, s * XIN_SET, [(XIN_G, G), (1, T * D)]),
                    in_=xr[n],
                ).then_inc(sems["dma_in0" if s == 0 else "dma_in1"], 16)
                if n >= 2:
                    store(n - 2)
            store(NT - 2)
            store(NT - 1)
            sync.wait_ge(sems["out0"], 16 * ((NT + 1) // 2))
            sync.wait_ge(sems["out1"], 16 * (NT // 2))

        @block.scalar
        def _(scalar):
            Copy = mybir.ActivationFunctionType.Copy
            Exp = mybir.ActivationFunctionType.Exp

            def prep(n):
                s = n % 2
                scalar.wait_ge(sems["dma_in0" if s == 0 else "dma_in1"],
                               16 * (n // 2 + 1))
                # xt16[g][d][t] = x[g][t][d]
                scalar.activation(
                    out=_ap(xt16, s * XT_SET, [(XT_G, G), (T, D), (1, T)]),
                    in_=_ap(xin, s * XIN_SET, [(XIN_G, G), (1, D), (D, T)]),
                    func=Copy)
                # xpb[a][g][i][j] = xp[g][i][a]  (AB: all i, j<18; C: i,j>=split)
                for a in range(POS):
                    act = scalar.activation(
                        out=_ap(xpb, s * XPB_SET + a * XPB_A,
                                [(XPB_G, G), (JH, T), (1, JH)]),
                        in_=_ap(xin, s * XIN_SET + POS + a,
                                [(XIN_G, G), (D, T), (0, JH)]),
                        func=Copy)
                    if a == POS - 1:
                        act.then_inc(sems["xin_done"], 1)

            def expf(n):
                s = n % 2
                scalar.wait_ge(sems["s_done"], n + 1)
                if n >= 2:
                    scalar.wait_ge(sems["b_done"], n - 1)
                # AB: all i, j<18 ((g,h) merged: TS_G == NH*TS_H)
                scalar.activation(
                    out=_ap(t1, s * TS_SET, [(TS_H, G * NH), (J, T), (1, JH)]),
                    in_=_ap(t0, s * TS_SET, [(TS_H, G * NH), (J, T), (1, JH)]),
                    func=Exp)
                # C: i>=17, j>=18
                scalar.activation(
                    out=_ap(t1, s * TS_SET + H * J + JH,
                            [(TS_H, G * NH), (J, H), (1, JH)]),
                    in_=_ap(t0, s * TS_SET + H * J + JH,
                            [(TS_H, G * NH), (J, H), (1, JH)]),
                    func=Exp,
                ).then_inc(sems["e_done"], 1)

            scalar.wait_ge(sems["const"], 16)
            prep(0)
            prep(1)
            for n in range(NT):
                expf(n)
                if n + 2 < NT:
                    prep(n + 2)

        @block.vector
        def _(vector):
            vector.wait_ge(sems["const"], 16)
            # zero the j=34,35 pad columns of yv once (proj never writes
            # them; PV/scores read them; mask kills their contribution but
            # they must be finite)
            vector.memset(
                _ap(yv, T, [(J, 2 * 2 * G * D), (1, J - T)]), 0.0)

            def phase_a(n):
                s = n % 2
                # xin_done is incremented by ACT prep(n)'s last copy, which
                # also certifies xt16/xpb for this tile
                vector.wait_ge(sems["xin_done"], n + 1)
                # projections: yv[w][g][hc][j] = sum_b xt16[g][3(1-w)+b][j] * A2[w][hc][b]
                for w in range(2):
                    for b in range(POS):
                        i0 = _ap(xt16, s * XT_SET + (3 * (1 - w) + b) * T,
                                 [(XT_G, G), (0, D), (1, T)])
                        i1 = _ap(wsh, OFFH_A2 + (w * POS + b) * D * T,
                                 [(0, G), (T, D), (1, T)])
                        if b == 0:
                            vector.tensor_mul(
                                out=_ap(yv, s * YV_SET + w * YV_W,
                                        [(YV_G, G), (J, D), (1, T)]),
                                in0=i0, in1=i1)
                        else:
                            vector.tensor_mul(
                                out=_ap(tmp, 0, [(D * J, G), (J, D), (1, T)]),
                                in0=i0, in1=i1)
                            vector.tensor_add(
                                out=_ap(yv, s * YV_SET + w * YV_W,
                                        [(YV_G, G), (J, D), (1, T)]),
                                in0=_ap(yv, s * YV_SET + w * YV_W,
                                        [(YV_G, G), (J, D), (1, T)]),
                                in1=_ap(tmp, 0, [(D * J, G), (J, D), (1, T)]))
                # scores (smul split per head: xpb has no h dim to merge)
                def smul(a, dst):
                    for h in range(NH):
                        # AB: all i, j<18
                        vector.tensor_mul(
                            out=_ap(dst, s * TS_SET + h * TS_H,
                                    [(TS_G, G), (J, T), (1, JH)]),
                            in0=_ap(xpb, s * XPB_SET + a * XPB_A,
                                    [(XPB_G, G), (JH, T), (1, JH)]),
                            in1=_ap(yv, s * YV_SET + (h * HD + a) * J,
                                    [(YV_G, G), (0, T), (1, JH)]))
                        # C: i>=17, j>=18 (xpb cols 0:18 hold the same xp)
                        vector.tensor_mul(
                            out=_ap(dst, s * TS_SET + h * TS_H + H * J + JH,
                                    [(TS_G, G), (J, H), (1, JH)]),
                            in0=_ap(xpb, s * XPB_SET + a * XPB_A + H * JH,
                                    [(XPB_G, G), (JH, H), (1, JH)]),
                            in1=_ap(yv, s * YV_SET + (h * HD + a) * J + JH,
                                    [(YV_G, G), (0, H), (1, JH)]))

                def tadd(region):
                    li = T if region == "AB" else H
                    off = 0 if region == "AB" else H * J + JH
                    vector.tensor_add(
                        out=_ap(t0, s * TS_SET + off,
                                [(TS_H, G * NH), (J, li), (1, JH)]),
                        in0=_ap(t0, s * TS_SET + off,
                                [(TS_H, G * NH), (J, li), (1, JH)]),
                        in1=_ap(t1, s * TS_SET + off,
                                [(TS_H, G * NH), (J, li), (1, JH)]))

                smul(0, t0)
                smul(1, t1)
                tadd("AB")
                tadd("C")
                smul(2, t1)
                # mask: t1[i<17, j<18] += maskA ; t1[C] += maskC
                vector.tensor_add(
                    out=_ap(t1, s * TS_SET, [(TS_H, G * NH), (J, H), (1, JH)]),
                    in0=_ap(t1, s * TS_SET, [(TS_H, G * NH), (J, H), (1, JH)]),
                    in1=_ap(wsh, OFFH_MA, [(0, G * NH), (JH, H), (1, JH)]))
                vector.tensor_add(
                    out=_ap(t1, s * TS_SET + H * J + JH,
                            [(TS_H, G * NH), (J, H), (1, JH)]),
                    in0=_ap(t1, s * TS_SET + H * J + JH,
                            [(TS_H, G * NH), (J, H), (1, JH)]),
                    in1=_ap(wsh, OFFH_MC, [(0, G * NH), (JH, H), (1, JH)]))
                tadd("AB")
                vector.tensor_add(
                    out=_ap(t0, s * TS_SET + H * J + JH,
                            [(TS_H, G * NH), (J, H), (1, JH)]),
                    in0=_ap(t0, s * TS_SET + H * J + JH,
                            [(TS_H, G * NH), (J, H), (1, JH)]),
                    in1=_ap(t1, s * TS_SET + H * J + JH,
                            [(TS_H, G * NH), (J, H), (1, JH)])
                ).then_inc(sems["s_done"], 1)

            def fold_chain(tsrc, soff, rows, into_ed):
                """rows i>=17: fold j[18:36) into j[0:18) (into ed or in
                place), then [16:18)->[0:2), [8:16)->[0:8)."""
                pass  # structured inline below for clarity

            def phase_b(n):
                s = n % 2
                vector.wait_ge(sems["e_done"], n + 1)
                if n >= 2:
                    vector.wait_ge(sems["out0" if s == 0 else "out1"],
                                   16 * (n // 2))
                # den: ed = e[i>=17, j<18] + e[i>=17, j>=18]  ((g,h) merged)
                vector.tensor_add(
                    out=_ap(ed, 0, [(ED_H, G * NH), (JH, H), (1, JH)]),
                    in0=_ap(t1, s * TS_SET + H * J,
                            [(TS_H, G * NH), (J, H), (1, JH)]),
                    in1=_ap(t1, s * TS_SET + H * J + JH,
                            [(TS_H, G * NH), (J, H), (1, JH)]))
                # fold [10:18) onto [0:8), then reduce the contiguous [0:10)
                vector.tensor_add(
                    out=_ap(ed, 0, [(ED_H, G * NH), (JH, H), (1, 8)]),
                    in0=_ap(ed, 0, [(ED_H, G * NH), (JH, H), (1, 8)]),
                    in1=_ap(ed, 10, [(ED_H, G * NH), (JH, H), (1, 8)]))
                vector.tensor_reduce(
                    out=_ap(den, 0, [(T, G * NH), (1, H)]),
                    in_=_ap(t1, s * TS_SET,
                            [(TS_H, G * NH), (J, H), (1, JH)]),
                    axis=mybir.AxisListType.X, op=mybir.AluOpType.add)
                vector.tensor_reduce(
                    out=_ap(den, H, [(T, G * NH), (1, H)]),
                    in_=_ap(ed, 0, [(ED_H, G * NH), (JH, H), (1, 10)]),
                    axis=mybir.AxisListType.X, op=mybir.AluOpType.add)
                vector.reciprocal(
                    out=_ap(rcp, 0, [(1, G * NH * T)]),
                    in_=_ap(den, 0, [(1, G * NH * T)]))
                # PV per channel c  (yv (g,h) merge: YV_G == NH*HD*J)
                for c in range(HD):
                    vector.tensor_mul(
                        out=_ap(pp, 0, [(PP_H, G * NH), (J, T), (1, JH)]),
                        in0=_ap(t1, s * TS_SET,
                                [(TS_H, G * NH), (J, T), (1, JH)]),
                        in1=_ap(yv, s * YV_SET + YV_W + c * J,
                                [(HD * J, G * NH), (0, T), (1, JH)]))
                    mm = vector.tensor_mul(
                        out=_ap(pp, H * J + JH,
                                [(PP_H, G * NH), (J, H), (1, JH)]),
                        in0=_ap(t1, s * TS_SET + H * J + JH,
                                [(TS_H, G * NH), (J, H), (1, JH)]),
                        in1=_ap(yv, s * YV_SET + YV_W + c * J + JH,
                                [(HD * J, G * NH), (0, H), (1, JH)]))
                    if c == HD - 1:
                        mm.then_inc(sems["b_done"], 1)
                    vector.tensor_add(
                        out=_ap(pp, H * J, [(PP_H, G * NH), (J, H), (1, JH)]),
                        in0=_ap(pp, H * J, [(PP_H, G * NH), (J, H), (1, JH)]),
                        in1=_ap(pp, H * J + JH,
                                [(PP_H, G * NH), (J, H), (1, JH)]))
                    vector.tensor_add(
                        out=_ap(pp, 0, [(PP_H, G * NH), (J, T), (1, 8)]),
                        in0=_ap(pp, 0, [(PP_H, G * NH), (J, T), (1, 8)]),
                        in1=_ap(pp, 10, [(PP_H, G * NH), (J, T), (1, 8)]))
                    vector.tensor_reduce(
                        out=_ap(o2, c, [(O2_G, G), (HD, NH), (D, T)]),
                        in_=_ap(pp, 0, [(PP_H, G * NH), (J, T), (1, 10)]),
                        axis=mybir.AxisListType.X, op=mybir.AluOpType.add)
                # normalize -> fp16 (split per head: rcp not (h,c)-mergeable)
                for h in range(NH):
                    vector.tensor_mul(
                        out=_ap(o2b, h * HD,
                                [(O2_G, G), (D, T), (1, HD)]),
                        in0=_ap(o2, h * HD,
                                [(O2_G, G), (D, T), (1, HD)]),
                        in1=_ap(rcp, h * T,
                                [(DEN_G, G), (1, T), (0, HD)]))
                # output projection (WoM replicated over t -> (t,e) merge)
                vector.tensor_mul(
                    out=_ap(prod, 0, [(PR_G, G), (T * D, D), (1, T * D)]),
                    in0=_ap(o2b, 0, [(O2_G, G), (0, D), (1, T * D)]),
                    in1=_ap(wsh, OFFH_WO, [(0, G), (T * D, D), (1, T * D)]))
                vector.tensor_reduce(
                    out=_ap(res, s * RES_SET, [(RES_SET // G, G), (1, D), (D, T)]),
                    in_=_ap(prod, 0, [(PR_G, G), (D, D * T), (1, D)]),
                    axis=mybir.AxisListType.X, op=mybir.AluOpType.add
                ).then_inc(sems["res_done"], 1)

            for n in range(NT):
                phase_a(n)
                if n >= 1:
                    phase_b(n - 1)
            phase_b(NT - 1)

    return nc


def _pack_weights(Wq, Wk, Wv, Wo):
    wth = np.zeros(HLEN, dtype=np.float16)
    scale = 1.0 / math.sqrt(HD)
    A2 = np.zeros((2, D, POS), dtype=np.float64)
    for h in range(NH):
        A2[0, h * HD:(h + 1) * HD, :] = (Wq[h * HD:(h + 1) * HD, :].T
                                         @ Wk[h * HD:(h + 1) * HD, :]) * scale
        A2[1, h * HD:(h + 1) * HD, :] = Wv[h * HD:(h + 1) * HD, :]
    # A2 replicated over t: [w][b][hc][t] = A2[w][hc][b]
    a2r = np.broadcast_to(
        A2.astype(np.float16).transpose(0, 2, 1)[:, :, :, None],
        (2, POS, D, T))
    wth[OFFH_A2:OFFH_A2 + 2 * POS * D * T] = a2r.reshape(-1)
    # WoM replicated over t: [dm][t][e] = Wo[dm][e]
    wom = np.broadcast_to(Wo.astype(np.float16)[:, None, :], (D, T, D))
    wth[OFFH_WO:OFFH_WO + D * T * D] = wom.reshape(-1)
    # maskA: rows i<17, cols j<18: -30000 where j > i
    ma = np.zeros((H, JH), dtype=np.float16)
    for i in range(H):
        ma[i, i + 1:] = MASKV
    # maskC: rows i'=i-17, cols j'=j-18: -30000 where j' >= i'  (covers pad)
    mc = np.zeros((H, JH), dtype=np.float16)
    for i in range(H):
        mc[i, i:] = MASKV
    wth[OFFH_MA:OFFH_MA + H * JH] = ma.reshape(-1)
    wth[OFFH_MC:OFFH_MC + H * JH] = mc.reshape(-1)
    return wth


@lru_cache(maxsize=2)
def _cached_kernel(bc, G):
    return build_kernel(bc, G)


def _prepare(x, Wq, Wk, Wv, Wo, G=2):
    x = np.ascontiguousarray(x, dtype=np.float32)
    B = x.shape[0]
    bc = B // NCORES
    nc = _cached_kernel(bc, G)
    wth = _pack_weights(np.asarray(Wq, dtype=np.float32),
                        np.asarray(Wk, dtype=np.float32),
                        np.asarray(Wv, dtype=np.float32),
                        np.asarray(Wo, dtype=np.float32))
    in_maps = [{"x": x[i * bc:(i + 1) * bc], "wth": wth}
               for i in range(NCORES)]
    return nc, in_maps


def kernel(x, Wq, Wk, Wv, Wo):
    nc, in_maps = _prepare(x, Wq, Wk, Wv, Wo)
    r = run_bass_kernel_spmd(nc, in_maps, core_ids=list(range(NCORES)))
    return np.concatenate([m["out"] for m in r.results], axis=0)


# revision 18
# speedup vs baseline: 1.6767x; 1.0416x over previous
"""Trainium2 Bass kernel for nn_Attn_40046275068166.

Tiny causal MHA over huge batch: x[B=65536, T=34, D=6], 2 heads, head_dim 3.
Pure data parallelism over 8 cores; batch on the 128 SBUF partitions inside
each core; per-example compute in the free dims.

v2 design notes (calibrated on HW probes):
- DVE tensor_tensor runs 2x ONLY when every operand is 2-byte, innermost
  stride +-1, 4B-aligned start, EVEN run length. Odd 17-runs degrade to
  ~0.8 ns/elem. So score planes are padded to J=36 columns and split at
  j=18; all fp16 ops use even runs.
- tensor_reduce is always 1x (~1.05 ns/elem) -> fold rows with 2x fp16
  adds (36->18->(8+2)) before reducing.
- GPSIMD tensor ops share SBUF ports with DVE and starve it 4.5x ->
  gpsimd does DMA orchestration ONLY.
- ACT (scalar engine) runs ~0.87 ns/elem on any strides; it materializes
  the xp broadcast planes (xpb) and the x transpose (xt16) so every DVE
  mul is all-fp16 stride-1, and computes exp.
- Weights are compile-time -> folded q/k projection A = Wq^T Wk / sqrt(hd);
  mask additive -30000 (fp16-safe, exp -> 0) covers the causal upper
  triangles AND the j=34,35 pad columns.

Pipeline: Pool streams x in / res out (SWDGE, parity-split semaphores);
ACT preps tile n+1 (xt16/xpb) then exps tile n; DVE does phase_a(n)
(proj+scores) then phase_b(n-1) (den/PV/out-projection).
"""

import math
from contextlib import ExitStack
from functools import lru_cache

import numpy as np

import concourse.bass as bass
from concourse import mybir
from concourse.bass_utils import run_bass_kernel_spmd

NCORES = 8
T = 34
J = 36              # padded score-plane width
H = 17              # i split
JH = 18             # j split
D = 6
NH = 2
HD = 3
POS = 3
P = 128

F32 = mybir.dt.float32
F16 = mybir.dt.float16

# fp16 constants layout (element offsets)
OFFH_MA = 0                 # maskA [17][18]
OFFH_MC = H * JH            # maskC [17][18]
OFFH_WO = 2 * H * JH        # WoM replicated [6][34][6] (WoM[dm][t][e]=Wo[dm][e])
OFFH_A2 = OFFH_WO + D * T * D   # A2 replicated [2][3(b)][6(hc)][34(t)]
HLEN = OFFH_A2 + 2 * POS * D * T

MASKV = -30000.0


def _ap(t, off, dims):
    p0 = t[:].ap[0]
    return bass.AP(tensor=t, offset=off, ap=[list(p0)] + [list(d) for d in dims])


def build_kernel(bc, G):
    assert bc % (P * G) == 0
    NT = bc // (P * G)

    nc = bass.Bass("TRN2")
    x = nc.dram_tensor("x", [bc, T, D], F32, kind="ExternalInput")
    wth = nc.dram_tensor("wth", [HLEN], F16, kind="ExternalInput")
    out = nc.dram_tensor("out", [bc, T, D], F32, kind="ExternalOutput")

    xr = x[:].rearrange("(n g p) t d -> n p g t d", g=G, p=P)
    outr = out[:].rearrange("(n g p) t d -> n p g t d", g=G, p=P)
    wth_b = bass.AP(tensor=wth, offset=0, ap=[[0, P], [1, HLEN]])

    # element strides
    XIN_SET = G * T * D         # 408
    XIN_G = T * D               # 34*6
    XT_SET = G * D * T          # xt16 [set][g][d][t]
    XT_G = D * T
    XPB_SET = POS * G * T * JH  # xpb [set][a][g][i][j<18] (C reuses cols 0:18)
    XPB_A = G * T * JH
    XPB_G = T * JH
    YV_SET = 2 * G * D * J      # yv [set][w][g][hc][j]
    YV_W = G * D * J
    YV_G = D * J
    TS_SET = G * NH * T * J     # t0/t1 [set][g][h][i][j]
    TS_G = NH * T * J
    TS_H = T * J
    PP_G = NH * T * J           # pp [g][h][i][j]
    PP_H = T * J
    ED_G = NH * H * JH          # ed [g][h][i'][j']
    ED_H = H * JH
    DEN_G = NH * T
    O2_G = T * D
    PR_G = D * T * D            # prod [g][dm][t][e]
    RES_SET = G * T * D

    with ExitStack() as ctx:
        sb = lambda nm, shape, dt=F32: ctx.enter_context(
            nc.sbuf_tensor(nm, shape, dt))
        wsh = sb("wsh", [P, HLEN], F16)
        xin = sb("xin", [P, 2, G, T, D])
        xt16 = sb("xt16", [P, 2, G, D, T], F16)
        xpb = sb("xpb", [P, 2, POS, G, T, JH], F16)
        yv = sb("yv", [P, 2, 2, G, D, J], F16)
        t0 = sb("t0", [P, 2, G, NH, T, J], F16)
        t1 = sb("t1", [P, 2, G, NH, T, J], F16)
        tmp = sb("tmp", [P, G, D, J], F16)
        ed = sb("ed", [P, G, NH, H, JH], F16)
        pp = sb("pp", [P, G, NH, T, J], F16)
        den = sb("den", [P, G, NH, T])
        rcp = sb("rcp", [P, G, NH, T])
        o2 = sb("o2", [P, G, T, D])
        o2b = sb("o2b", [P, G, T, D], F16)
        prod = sb("prod", [P, G, D, T, D], F16)
        res = sb("res", [P, 2, G, T, D])

        sem_names = ["dma_in0", "dma_in1", "const", "xin_done", "prep_done",
                     "s_done", "e_done", "b_done", "res_done", "out0", "out1"]
        sems = {k: ctx.enter_context(nc.semaphore(name=k)) for k in sem_names}

        block = ctx.enter_context(nc.Block())

        @block.gpsimd
        def _(sync):
            # DMA orchestration ONLY (gpsimd tensor ops would starve DVE's
            # SBUF ports). SWDGE: one +16 completion inc per dma_start.
            def store(k):
                sp = k % 2
                sync.wait_ge(sems["res_done"], k + 1)
                sync.dma_start(
                    out=outr[k],
                    in_=_ap(res, sp * RES_SET, [(XIN_G, G), (1, T * D)]),
                ).then_inc(sems["out0" if sp == 0 else "out1"], 16)

            sync.dma_start(out=wsh[:], in_=wth_b).then_inc(sems["const"], 16)
            for n in range(NT):
                s = n % 2
                if n >= 2:
                    sync.wait_ge(sems["xin_done"], n - 1)
                sync.dma_start(
                    out=_ap(xin, s * XIN_SET, [(XIN_G, G), (1, T * D)]),
                    in_=xr[n],
                ).then_inc(sems["dma_in0" if s == 0 else "dma_in1"], 16)
                if n >= 2:
                    store(n - 2)
            store(NT - 2)
            store(NT - 1)
            sync.wait_ge(sems["out0"], 16 * ((NT + 1) // 2))
            sync.wait_ge(sems["out1"], 16 * (NT // 2))

        @block.scalar
        def _(scalar):
            Copy = mybir.ActivationFunctionType.Copy
            Exp = mybir.ActivationFunctionType.Exp

            def prep(n):
                s = n % 2
                scalar.wait_ge(sems["dma_in0" if s == 0 else "dma_in1"],
                               16 * (n // 2 + 1))
                # xt16[g][d][t] = x[g][t][d]
                scalar.activation(
                    out=_ap(xt16, s * XT_SET, [(XT_G, G), (T, D), (1, T)]),
                    in_=_ap(xin, s * XIN_SET, [(XIN_G, G), (1, D), (D, T)]),
                    func=Copy)
                # xpb[a][g][i][j] = xp[g][i][a]  (AB: all i, j<18; C: i,j>=split)
                for a in range(POS):
                    act = scalar.activation(
                        out=_ap(xpb, s * XPB_SET + a * XPB_A,
                                [(XPB_G, G), (JH, T), (1, JH)]),
                        in_=_ap(xin, s * XIN_SET + POS + a,
                                [(XIN_G, G), (D, T), (0, JH)]),
                        func=Copy)
                    if a == POS - 1:
                        act.then_inc(sems["xin_done"], 1)

            def expf(n):
                s = n % 2
                scalar.wait_ge(sems["s_done"], n + 1)
                if n >= 2:
                    scalar.wait_ge(sems["b_done"], n - 1)
                # AB: all i, j<18 ((g,h) merged: TS_G == NH*TS_H)
                scalar.activation(
                    out=_ap(t1, s * TS_SET, [(TS_H, G * NH), (J, T), (1, JH)]),
                    in_=_ap(t0, s * TS_SET, [(TS_H, G * NH), (J, T), (1, JH)]),
                    func=Exp)
                # C: i>=17, j>=18
                scalar.activation(
                    out=_ap(t1, s * TS_SET + H * J + JH,
                            [(TS_H, G * NH), (J, H), (1, JH)]),
                    in_=_ap(t0, s * TS_SET + H * J + JH,
                            [(TS_H, G * NH), (J, H), (1, JH)]),
                    func=Exp,
                ).then_inc(sems["e_done"], 1)

            scalar.wait_ge(sems["const"], 16)
            prep(0)
            prep(1)
            for n in range(NT):
                expf(n)
                if n + 2 < NT:
                    prep(n + 2)

        @block.vector
        def _(vector):
            vector.wait_ge(sems["const"], 16)
            # zero the j=34,35 pad columns of yv once (proj never writes
            # them; PV/scores read them; mask kills their contribution but
            # they must be finite)
            vector.memset(
                _ap(yv, T, [(J, 2 * 2 * G * D), (1, J - T)]), 0.0)

            def phase_a(n):
                s = n % 2
                # xin_done is incremented by ACT prep(n)'s last copy, which
                # also certifies xt16/xpb for this tile
                vector.wait_ge(sems["xin_done"], n + 1)
                # projections: yv[w][g][hc][j] = sum_b xt16[g][3(1-w)+b][j] * A2[w][hc][b]
                for w in range(2):
                    for b in range(POS):
                        i0 = _ap(xt16, s * XT_SET + (3 * (1 - w) + b) * T,
                                 [(XT_G, G), (0, D), (1, T)])
                        i1 = _ap(wsh, OFFH_A2 + (w * POS + b) * D * T,
                                 [(0, G), (T, D), (1, T)])
                        if b == 0:
                            vector.tensor_mul(
                                out=_ap(yv, s * YV_SET + w * YV_W,
                                        [(YV_G, G), (J, D), (1, T)]),
                                in0=i0, in1=i1)
                        else:
                            vector.tensor_mul(
                                out=_ap(tmp, 0, [(D * J, G), (J, D), (1, T)]),
                                in0=i0, in1=i1)
                            vector.tensor_add(
                                out=_ap(yv, s * YV_SET + w * YV_W,
                                        [(YV_G, G), (J, D), (1, T)]),
                                in0=_ap(yv, s * YV_SET + w * YV_W,
                                        [(YV_G, G), (J, D), (1, T)]),
                                in1=_ap(tmp, 0, [(D * J, G), (J, D), (1, T)]))
                # scores (smul split per head: xpb has no h dim to merge)
                def smul(a, dst):
                    for h in range(NH):
                        # AB: all i, j<18
                        vector.tensor_mul(
                            out=_ap(dst, s * TS_SET + h * TS_H,
                                    [(TS_G, G), (J, T), (1, JH)]),
                            in0=_ap(xpb, s * XPB_SET + a * XPB_A,
                                    [(XPB_G, G), (JH, T), (1, JH)]),
                            in1=_ap(yv, s * YV_SET + (h * HD + a) * J,
                                    [(YV_G, G), (0, T), (1, JH)]))
                        # C: i>=17, j>=18 (xpb cols 0:18 hold the same xp)
                        vector.tensor_mul(
                            out=_ap(dst, s * TS_SET + h * TS_H + H * J + JH,
                                    [(TS_G, G), (J, H), (1, JH)]),
                            in0=_ap(xpb, s * XPB_SET + a * XPB_A + H * JH,
                                    [(XPB_G, G), (JH, H), (1, JH)]),
                            in1=_ap(yv, s * YV_SET + (h * HD + a) * J + JH,
                                    [(YV_G, G), (0, H), (1, JH)]))

                def tadd(region):
                    li = T if region == "AB" else H
                    off = 0 if region == "AB" else H * J + JH
                    vector.tensor_add(
                        out=_ap(t0, s * TS_SET + off,
                                [(TS_H, G * NH), (J, li), (1, JH)]),
                        in0=_ap(t0, s * TS_SET + off,
                                [(TS_H, G * NH), (J, li), (1, JH)]),
                        in1=_ap(t1, s * TS_SET + off,
                                [(TS_H, G * NH), (J, li), (1, JH)]))

                smul(0, t0)
                smul(1, t1)
                tadd("AB")
                tadd("C")
                smul(2, t1)
                # mask: t1[i<17, j<18] += maskA ; t1[C] += maskC
                vector.tensor_add(
                    out=_ap(t1, s * TS_SET, [(TS_H, G * NH), (J, H), (1, JH)]),
                    in0=_ap(t1, s * TS_SET, [(TS_H, G * NH), (J, H), (1, JH)]),
                    in1=_ap(wsh, OFFH_MA, [(0, G * NH), (JH, H), (1, JH)]))
                vector.tensor_add(
                    out=_ap(t1, s * TS_SET + H * J + JH,
                            [(TS_H, G * NH), (J, H), (1, JH)]),
                    in0=_ap(t1, s * TS_SET + H * J + JH,
                            [(TS_H, G * NH), (J, H), (1, JH)]),
                    in1=_ap(wsh, OFFH_MC, [(0, G * NH), (JH, H), (1, JH)]))
                tadd("AB")
                vector.tensor_add(
                    out=_ap(t0, s * TS_SET + H * J + JH,
                            [(TS_H, G * NH), (J, H), (1, JH)]),
                    in0=_ap(t0, s * TS_SET + H * J + JH,
                            [(TS_H, G * NH), (J, H), (1, JH)]),
                    in1=_ap(t1, s * TS_SET + H * J + JH,
                            [(TS_H, G * NH), (J, H), (1, JH)])
                ).then_inc(sems["s_done"], 1)

            def fold_chain(tsrc, soff, rows, into_ed):
                """rows i>=17: fold j[18:36) into j[0:18) (into ed or in
                place), then [16:18)->[0:2), [8:16)->[0:8)."""
                pass  # structured inline below for clarity

            def phase_b(n):
                s = n % 2
                vector.wait_ge(sems["e_done"], n + 1)
                if n >= 2:
                    vector.wait_ge(sems["out0" if s == 0 else "out1"],
                                   16 * (n // 2))
                # den: ed = e[i>=17, j<18] + e[i>=17, j>=18]  ((g,h) merged)
                vector.tensor_add(
                    out=_ap(ed, 0, [(ED_H, G * NH), (JH, H), (1, JH)]),
                    in0=_ap(t1, s * TS_SET + H * J,
                            [(TS_H, G * NH), (J, H), (1, JH)]),
                    in1=_ap(t1, s * TS_SET + H * J + JH,
                            [(TS_H, G * NH), (J, H), (1, JH)]))
                # fold [10:18) onto [0:8), then reduce the contiguous [0:10)
                vector.tensor_add(
                    out=_ap(ed, 0, [(ED_H, G * NH), (JH, H), (1, 8)]),
                    in0=_ap(ed, 0, [(ED_H, G * NH), (JH, H), (1, 8)]),
                    in1=_ap(ed, 10, [(ED_H, G * NH), (JH, H), (1, 8)]))
                vector.tensor_reduce(
                    out=_ap(den, 0, [(T, G * NH), (1, H)]),
                    in_=_ap(t1, s * TS_SET,
                            [(TS_H, G * NH), (J, H), (1, JH)]),
                    axis=mybir.AxisListType.X, op=mybir.AluOpType.add)
                vector.tensor_reduce(
                    out=_ap(den, H, [(T, G * NH), (1, H)]),
                    in_=_ap(ed, 0, [(ED_H, G * NH), (JH, H), (1, 10)]),
                    axis=mybir.AxisListType.X, op=mybir.AluOpType.add)
                vector.reciprocal(
                    out=_ap(rcp, 0, [(1, G * NH * T)]),
                    in_=_ap(den, 0, [(1, G * NH * T)]))
                # PV per channel c  (yv (g,h) merge: YV_G == NH*HD*J)
                for c in range(HD):
                    vector.tensor_mul(
                        out=_ap(pp, 0, [(PP_H, G * NH), (J, T), (1, JH)]),
                        in0=_ap(t1, s * TS_SET,
                                [(TS_H, G * NH), (J, T), (1, JH)]),
                        in1=_ap(yv, s * YV_SET + YV_W + c * J,
                                [(HD * J, G * NH), (0, T), (1, JH)]))
                    mm = vector.tensor_mul(
                        out=_ap(pp, H * J + JH,
                                [(PP_H, G * NH), (J, H), (1, JH)]),
                        in0=_ap(t1, s * TS_SET + H * J + JH,
                                [(TS_H, G * NH), (J, H), (1, JH)]),
                        in1=_ap(yv, s * YV_SET + YV_W + c * J + JH,
                                [(HD * J, G * NH), (0, H), (1, JH)]))
                    if c == HD - 1:
                        mm.then_inc(sems["b_done"], 1)
                    vector.tensor_add(
                        out=_ap(pp, H * J, [(PP_H, G * NH), (J, H), (1, JH)]),
                        in0=_ap(pp, H * J, [(PP_H, G * NH), (J, H), (1, JH)]),
                        in1=_ap(pp, H * J + JH,
                                [(PP_H, G * NH), (J, H), (1, JH)]))
                    vector.tensor_add(
                        out=_ap(pp, 0, [(PP_H, G * NH), (J, T), (1, 8)]),
                        in0=_ap(pp, 0, [(PP_H, G * NH), (J, T), (1, 8)]),
                        in1=_ap(pp, 10, [(PP_H, G * NH), (J, T), (1, 8)]))
                    vector.tensor_reduce(
                        out=_ap(o2, c, [(O2_G, G), (HD, NH), (D, T)]),
                        in_=_ap(pp, 0, [(PP_H, G * NH), (J, T), (1, 10)]),
                        axis=mybir.AxisListType.X, op=mybir.AluOpType.add)
                # normalize -> fp16 (split per head: rcp not (h,c)-mergeable)
                for h in range(NH):
                    vector.tensor_mul(
                        out=_ap(o2b, h * HD,
                                [(O2_G, G), (D, T), (1, HD)]),
                        in0=_ap(o2, h * HD,
                                [(O2_G, G), (D, T), (1, HD)]),
                        in1=_ap(rcp, h * T,
                                [(DEN_G, G), (1, T), (0, HD)]))
                # output projection (WoM replicated over t -> (t,e) merge)
                vector.tensor_mul(
                    out=_ap(prod, 0, [(PR_G, G), (T * D, D), (1, T * D)]),
                    in0=_ap(o2b, 0, [(O2_G, G), (0, D), (1, T * D)]),
                    in1=_ap(wsh, OFFH_WO, [(0, G), (T * D, D), (1, T * D)]))
                vector.tensor_reduce(
                    out=_ap(res, s * RES_SET, [(RES_SET // G, G), (1, D), (D, T)]),
                    in_=_ap(prod, 0, [(PR_G, G), (D, D * T), (1, D)]),
                    axis=mybir.AxisListType.X, op=mybir.AluOpType.add
                ).then_inc(sems["res_done"], 1)

            for n in range(NT):
                phase_a(n)
                if n >= 1:
                    phase_b(n - 1)
            phase_b(NT - 1)

    return nc


def _pack_weights(Wq, Wk, Wv, Wo):
    wth = np.zeros(HLEN, dtype=np.float16)
    scale = 1.0 / math.sqrt(HD)
    A2 = np.zeros((2, D, POS), dtype=np.float64)
    for h in range(NH):
        A2[0, h * HD:(h + 1) * HD, :] = (Wq[h * HD:(h + 1) * HD, :].T
                                         @ Wk[h * HD:(h + 1) * HD, :]) * scale
        A2[1, h * HD:(h + 1) * HD, :] = Wv[h * HD:(h + 1) * HD, :]
    # A2 replicated over t: [w][b][hc][t] = A2[w][hc][b]
    a2r = np.broadcast_to(
        A2.astype(np.float16).transpose(0, 2, 1)[:, :, :, None],
        (2, POS, D, T))
    wth[OFFH_A2:OFFH_A2 + 2 * POS * D * T] = a2r.reshape(-1)
    # WoM replicated over t: [dm][t][e] = Wo[dm][e]
    wom = np.broadcast_to(Wo.astype(np.float16)[:, None, :], (D, T, D))
    wth[OFFH_WO:OFFH_WO + D * T * D] = wom.reshape(-1)
    # maskA: rows i<17, cols j<18: -30000 where j > i
    ma = np.zeros((H, JH), dtype=np.float16)
    for i in range(H):
        ma[i, i + 1:] = MASKV
    # maskC: rows i'=i-17, cols j'=j-18: -30000 where j' >= i'  (covers pad)
    mc = np.zeros((H, JH), dtype=np.float16)
    for i in range(H):
        mc[i, i:] = MASKV
    wth[OFFH_MA:OFFH_MA + H * JH] = ma.reshape(-1)
    wth[OFFH_MC:OFFH_MC + H * JH] = mc.reshape(-1)
    return wth


@lru_cache(maxsize=2)
def _cached_kernel(bc, G):
    return build_kernel(bc, G)


def _prepare(x, Wq, Wk, Wv, Wo, G=4):
    x = np.ascontiguousarray(x, dtype=np.float32)
    B = x.shape[0]
    bc = B // NCORES
    nc = _cached_kernel(bc, G)
    wth = _pack_weights(np.asarray(Wq, dtype=np.float32),
                        np.asarray(Wk, dtype=np.float32),
                        np.asarray(Wv, dtype=np.float32),
                        np.asarray(Wo, dtype=np.float32))
    in_maps = [{"x": x[i * bc:(i + 1) * bc], "wth": wth}
               for i in range(NCORES)]
    return nc, in_maps


def kernel(x, Wq, Wk, Wv, Wo):
    nc, in_maps = _prepare(x, Wq, Wk, Wv, Wo)
    r = run_bass_kernel_spmd(nc, in_maps, core_ids=list(range(NCORES)))
    return np.concatenate([m["out"] for m in r.results], axis=0)
